# revision 1
# baseline (speedup 1.0000x reference)
"""Trainium2 Bass kernel for nn_Cross_Attention (sparse_attention, 8 cores).

Self-contained: builds two SPMD Bass programs (L1 convs+gram partials, L2 attn-apply),
shards 4 samples x 2 row-halves across 8 NeuronCores, glues partials on host.
"""
import sys
sys.path.insert(0, "/opt/trn_rl_repo")
import numpy as np
import ml_dtypes

import concourse.bass as bass
import concourse.tile as tile
from concourse import bacc, mybir
from contextlib import ExitStack




BF16 = mybir.dt.bfloat16
F32 = mybir.dt.float32
bf16 = ml_dtypes.bfloat16

TAPS = [(dy, dx) for dy in (-1, 0, 1) for dx in (-1, 0, 1)]


def qkv_halves():
    """Per (pb, half): (x1_base, qkv_base, ch0, nch).  ch0 = qkv-global channel."""
    out = []
    for pb in range(6):
        P, odd = pb // 2, pb % 2
        for h in (0, 1):
            nch = 3 if (odd and h == 1) else 63
            ch0 = 3 * (64 * P + 42 * odd + 21 * h)
            x1b = 64 * h
            qb = 64 * h if not odd else 64 * (1 - h)
            out.append((pb, h, x1b, qb, ch0, nch))
    return out


def qkv_channel_at(pb, p):
    """qkv-global channel stored at partition p of qkv pblock pb, or None."""
    for (pb2, h, x1b, qb, ch0, nch) in qkv_halves():
        if pb2 == pb and qb <= p < qb + nch:
            return ch0 + (p - qb)
    return None


# newk/newv input chunks: (source, pb, base, size)
KCC = [("qkv", 2, 0, 128), ("qkv", 3, 0, 128), ("dw", 0, 0, 128), ("dw", 1, 64, 64)]
VCC = [("qkv", 4, 0, 128), ("qkv", 5, 0, 128), ("dw", 1, 0, 64), ("dw", 2, 0, 128)]
# kvdw 64-blocks: (in_pb, in_base, out_pb, out_base); dw pb1 halves swapped
DWBLK = [(0, 0, 0, 0), (0, 64, 0, 64), (1, 0, 1, 64),
         (1, 64, 1, 0), (2, 0, 2, 0), (2, 64, 2, 64)]


def _bcast(ap, p):
    return bass.AP(tensor=ap.tensor, offset=ap.offset, ap=[[0, p]] + list(ap.ap[1:]))


def build_l1(R=64, W=128, S=16):
    assert W == 128 and R % S == 0 and S % 4 == 0
    WP = W + 2  # 130
    NSLAB = R // S
    NS = S * W
    XCOLS = (S + 2) * WP
    MCOLS = (S + 4) * WP
    N128 = NS // 128

    nc = bacc.Bacc("TRN2", target_bir_lowering=False, debug=False, num_devices=8)

    def din(name, shape, dt=BF16):
        return nc.dram_tensor(name, shape, dt, kind="ExternalInput").ap()

    def dout(name, shape, dt=F32):
        return nc.dram_tensor(name, shape, dt, kind="ExternalOutput").ap()

    x_lo = din("x_lo", [128, (R + 2) * WP])
    x_hi = din("x_hi", [64, (R + 2) * WP])
    xm_lo = din("xm_lo", [128, (R + 4) * WP])
    xm_d1 = din("xm_d1", [128, (R + 4) * WP])
    xm_d2 = din("xm_d2", [128, (R + 4) * WP])
    qwT = din("qwT", [128, 2, 768])            # 12 zero-padded 64-ch windows
    qdw_wT = din("qdw_wT", [128, 9 * 6, 128])  # per (tap,pb): two 64x64 blocks
    kv_wT = din("kv_wT", [128, 2, 9, 384])
    kvp_wT = din("kvp_wT", [128, 4, 384])
    kvdw_wT = din("kvdw_wT", [128, 9 * 6, 64])
    newk_w_m = din("newk_w_m", [128, 5, 192])
    newv_wT = din("newv_wT", [128, 5, 192])
    ident = din("ident", [128, 128])
    ones_col = din("ones_col", [128, 1])
    x1_bias = din("x1_bias", [128, 6], F32)
    qkv_bias = din("qkv_bias", [128, 6], F32)
    kv_bias = din("kv_bias", [128, 3], F32)
    kvdw_bias = din("kvdw_bias", [128, 3], F32)
    newk_b_row = din("newk_b_row", [1, 192], F32)
    newv_bias = din("newv_bias", [128, 2], F32)
    mask_rc = din("mask_rc", [1, (R + 2) * WP])

    v_out = dout("v_out", [192, R * W], BF16)
    gram_out = dout("gram_out", [192, 256])
    kss_out = dout("kss_out", [1, 192])
    qstats_out = dout("qstats_out", [128, 2, 2])
    vstats_out = dout("vstats_out", [128, 2, 2])

    with tile.TileContext(nc) as tc, ExitStack() as ctx:
        wpool = ctx.enter_context(tc.tile_pool(name="weights", bufs=1))
        xpool = ctx.enter_context(tc.tile_pool(name="xslab", bufs=2))
        bigpool = ctx.enter_context(tc.tile_pool(name="big", bufs=1))
        midpool = ctx.enter_context(tc.tile_pool(name="mid", bufs=2))
        smpool = ctx.enter_context(tc.tile_pool(name="small", bufs=4))
        statpool = ctx.enter_context(tc.tile_pool(name="stats", bufs=1))
        pspool = ctx.enter_context(tc.tile_pool(name="ps", bufs=5, space="PSUM"))
        pspers = ctx.enter_context(tc.tile_pool(name="pspers", bufs=1, space="PSUM"))

        def load1(ap_in, shape, dt=BF16, eng=None):
            t = wpool.tile(shape, dt, tag=ap_in.tensor.name)
            (eng or nc.sync).dma_start(out=t[:ap_in.shape[0]], in_=ap_in[:])
            return t

        # phase-A-critical constants on the sync queue; the rest via gpsimd queue
        qwT_s = load1(qwT, [128, 2, 768])
        x1b_s = load1(x1_bias, [128, 6], F32)
        g = nc.gpsimd
        qdw_s = load1(qdw_wT, [128, 9 * 6, 128], eng=g)
        kvw_s = load1(kv_wT, [128, 2, 9, 384], eng=g)
        kvp_s = load1(kvp_wT, [128, 4, 384], eng=g)
        kvdw_s = load1(kvdw_wT, [128, 9 * 6, 64], eng=g)
        nkw_s = load1(newk_w_m, [128, 5, 192], eng=g)
        nvw_s = load1(newv_wT, [128, 5, 192], eng=g)
        id_s = load1(ident, [128, 128], eng=g)
        ones_s = load1(ones_col, [128, 1], eng=g)
        qkvb_s = load1(qkv_bias, [128, 6], F32, eng=g)
        kvb_s = load1(kv_bias, [128, 3], F32, eng=g)
        dwb_s = load1(kvdw_bias, [128, 3], F32, eng=g)
        nvb_s = load1(newv_bias, [128, 2], F32, eng=g)
        nkb_bc = wpool.tile([128, 192], F32, tag="nkb_bc")
        nc.gpsimd.dma_start(out=nkb_bc[:], in_=_bcast(newk_b_row[0:1, :], 128))

        gramA = pspers.tile([128, 256], F32)
        gramB = pspers.tile([64, 256], F32)
        kss_ps = pspers.tile([1, 192], F32)

        qstats = statpool.tile([128, 2, NSLAB * (NS // 512), 6], F32)
        vstats = statpool.tile([128, 2, NSLAB * (NS // 512), 6], F32)

        n128_total = NSLAB * N128

        for s in range(NSLAB):
            xsl_lo = xpool.tile([128, XCOLS], BF16, tag="xsl_lo")
            xsl_hi = xpool.tile([64, XCOLS], BF16, tag="xsl_hi")
            msl_lo = xpool.tile([128, MCOLS + 2], BF16, tag="msl_lo")
            msl_d1 = xpool.tile([128, MCOLS + 2], BF16, tag="msl_d1")
            msl_d2 = xpool.tile([128, MCOLS + 2], BF16, tag="msl_d2")
            off = s * S * WP
            nc.sync.dma_start(out=xsl_lo[:], in_=x_lo[:, off:off + XCOLS])
            nc.sync.dma_start(out=xsl_hi[:], in_=x_hi[:, off:off + XCOLS])
            nc.sync.dma_start(out=msl_lo[:, 1:1 + MCOLS], in_=xm_lo[:, off:off + MCOLS])
            nc.sync.dma_start(out=msl_d1[:, 1:1 + MCOLS], in_=xm_d1[:, off:off + MCOLS])
            nc.sync.dma_start(out=msl_d2[:, 1:1 + MCOLS], in_=xm_d2[:, off:off + MCOLS])
            for t in (msl_lo, msl_d1, msl_d2):
                nc.vector.memset(t[:, 0:1], 0.0)
                nc.vector.memset(t[:, MCOLS + 1:MCOLS + 2], 0.0)
            mtile = xpool.tile([128, XCOLS], BF16, tag="mtile")
            nc.sync.dma_start(out=mtile[:], in_=_bcast(mask_rc[0:1, off:off + XCOLS], 128))

            # ---- Phase A: x1 = 1x1(x), two col-tiled 64-ch windows per pblock
            x1 = bigpool.tile([128, 6, XCOLS], BF16, tag="x1")
            for pb in range(6):
                for c0 in range(0, XCOLS, 512):
                    cs = min(512, XCOLS - c0)
                    ps = pspool.tile([128, 512], F32, tag="ps", name=f"psA{s}_{pb}_{c0}")
                    for h in (0, 1):
                        for kb, (xin, ksz) in enumerate(((xsl_lo, 128), (xsl_hi, 64))):
                            nc.tensor.matmul(
                                ps[64 * h:64 * h + 64, :cs],
                                qwT_s[:ksz, kb, 128 * pb + 64 * h:128 * pb + 64 * h + 64],
                                xin[:, c0:c0 + cs], start=(kb == 0), stop=(kb == 1),
                                tile_position=(0, 64 * h), skip_group_check=True)
                    nc.vector.scalar_tensor_tensor(
                        out=x1[:, pb, c0:c0 + cs], in0=ps[:, :cs],
                        scalar=x1b_s[:, pb:pb + 1], in1=mtile[:, c0:c0 + cs],
                        op0=mybir.AluOpType.add, op1=mybir.AluOpType.mult)

            # ---- Phase C: kv1 = 3x3(xm), hi-ch taps packed via msl_d1/d2
            kv1 = bigpool.tile([128, 3, XCOLS], BF16, tag="kv1")
            for c0 in range(0, XCOLS, 512):
                cs = min(512, XCOLS - c0)
                for pb in range(3):
                    ps = pspool.tile([128, 512], F32, tag="ps", name=f"psC{s}_{pb}_{c0}")
                    passes = []
                    for ti, (dy, dx) in enumerate(TAPS):
                        passes.append((kvw_s[:128, 0, ti, :], msl_lo,
                                       c0 + (1 + dy) * WP + dx + 1))
                    for j, dx in enumerate((-1, 0, 1)):
                        passes.append((kvp_s[:, j, :], msl_d1, c0 + dx + 1))
                    passes.append((kvp_s[:, 3, :], msl_d2, c0 + 2 * WP))
                    passes.append((kvw_s[:64, 1, 7, :], msl_d1, c0 + 2 * WP + 1))
                    nmm = len(passes)
                    for i, (wpl, xin, moff) in enumerate(passes):
                        ksz = wpl.shape[0]
                        xa = xin[:ksz, moff:moff + cs]
                        if pb == 1:
                            nc.tensor.matmul(ps[0:64, :cs], wpl[:, 128:192], xa,
                                             start=(i == 0), stop=(i == nmm - 1),
                                             tile_position=(0, 0), skip_group_check=True)
                            nc.tensor.matmul(ps[64:128, :cs], wpl[:, 192:256], xa,
                                             start=(i == 0), stop=(i == nmm - 1),
                                             tile_position=(0, 64), skip_group_check=True)
                        else:
                            msl_ = wpl[:, 0:128] if pb == 0 else wpl[:, 256:384]
                            nc.tensor.matmul(ps[:, :cs], msl_, xa,
                                             start=(i == 0), stop=(i == nmm - 1))
                    nc.vector.scalar_tensor_tensor(
                        out=kv1[:, pb, c0:c0 + cs], in0=ps[:, :cs],
                        scalar=kvb_s[:, pb:pb + 1], in1=mtile[:, c0:c0 + cs],
                        op0=mybir.AluOpType.add, op1=mybir.AluOpType.mult)

            # ---- Phase B: qkv = qdw(x1): pblock pairs, 4 concurrent 64x64 blocks
            qkv = bigpool.tile([128, 6, NS], BF16, tag="qkv")
            x1v = [x1[:, pb, :].rearrange("p (r c) -> p r c", c=WP) for pb in range(6)]
            for sset in range(3):
                for ic in range(S // 4):
                    r0 = ic * 4
                    pse = pspool.tile([128, 512], F32, tag="ps", name=f"psBe{s}_{sset}_{ic}")
                    pso = pspool.tile([128, 512], F32, tag="ps", name=f"psBo{s}_{sset}_{ic}")
                    for ti, (dy, dx) in enumerate(TAPS):
                        for j in (0, 1):
                            pb = 2 * sset + j
                            pst = pse if j == 0 else pso
                            for h in (0, 1):
                                qb = 64 * h if j == 0 else 64 * (1 - h)
                                rhs = x1v[pb][64 * h:64 * h + 64,
                                              r0 + 1 + dy:r0 + 5 + dy, 1 + dx:1 + dx + 128]
                                nc.tensor.matmul(
                                    pst[qb:qb + 64, :],
                                    qdw_s[64 * h:64 * h + 64, 6 * ti + pb, qb:qb + 64],
                                    rhs, start=(ti == 0), stop=(ti == 8),
                                    tile_position=(64 * h, qb), skip_group_check=True)
                    for j in (0, 1):
                        pb = 2 * sset + j
                        nc.scalar.activation(out=qkv[:, pb, r0 * 128:(r0 + 4) * 128],
                                             in_=(pse if j == 0 else pso)[:, :],
                                             func=mybir.ActivationFunctionType.Identity,
                                             bias=qkvb_s[:, pb:pb + 1], scale=1.0)

            # ---- Phase D: kvdw via 6 concurrent 64x64 diag blocks
            dw = bigpool.tile([128, 3, NS], BF16, tag="dw")
            kv1v = [kv1[:, pb, :].rearrange("p (r c) -> p r c", c=WP) for pb in range(3)]
            for ic in range(S // 4):
                r0 = ic * 4
                pss = [pspool.tile([128, 512], F32, tag="ps", name=f"psD{s}_{ic}_{i}")
                       for i in range(3)]
                for ti, (dy, dx) in enumerate(TAPS):
                    for blk, (ipb, ib, opb, ob) in enumerate(DWBLK):
                        rhs = kv1v[ipb][ib:ib + 64,
                                        r0 + 1 + dy:r0 + 5 + dy, 1 + dx:1 + dx + 128]
                        nc.tensor.matmul(pss[opb][ob:ob + 64, :],
                                         kvdw_s[ib:ib + 64, 6 * ti + blk, :],
                                         rhs, start=(ti == 0), stop=(ti == 8),
                                         tile_position=(ib, ob), skip_group_check=True)
                for pb in range(3):
                    nc.scalar.activation(out=dw[:, pb, r0 * 128:(r0 + 4) * 128],
                                         in_=pss[pb][:, :],
                                         func=mybir.ActivationFunctionType.Identity,
                                         bias=dwb_s[:, pb:pb + 1], scale=1.0)

            # ---- Phase E: v = newv(v_cc) + stats + dma out
            vt = midpool.tile([128, 2, NS], BF16, tag="vt")
            for ic in range(NS // 512):
                c0 = ic * 512
                for mb in range(2):
                    msz = 128 if mb == 0 else 64
                    ps = pspool.tile([128, 512], F32, tag="ps", name=f"psE{s}_{ic}_{mb}")
                    for j, (src, pb, base, sz) in enumerate(VCC):
                        data = (qkv if src == "qkv" else dw)
                        rhs = data[base:base + sz, pb, c0:c0 + 512]
                        lhsT = nvw_s[base:base + sz, j, mb * 128:mb * 128 + msz]
                        nc.tensor.matmul(ps[:msz, :], lhsT, rhs, start=(j == 0),
                                         stop=(j == 3),
                                         tile_position=(base, 0) if base else None)
                    nc.scalar.activation(out=vt[:msz, mb, c0:c0 + 512], in_=ps[:msz, :],
                                         func=mybir.ActivationFunctionType.Identity,
                                         bias=nvb_s[:msz, mb:mb + 1], scale=1.0)
            nc.sync.dma_start(out=v_out[0:128, s * NS:(s + 1) * NS], in_=vt[:, 0, :])
            nc.sync.dma_start(out=v_out[128:192, s * NS:(s + 1) * NS], in_=vt[:64, 1, :])
            for sub in range(NS // 512):
                si = s * (NS // 512) + sub
                sl = slice(sub * 512, (sub + 1) * 512)
                nc.vector.bn_stats(out=vstats[:, 0, si, :], in_=vt[:, 0, sl])
                nc.vector.bn_stats(out=vstats[:64, 1, si, :], in_=vt[:64, 1, sl])
                nc.vector.bn_stats(out=qstats[:, 0, si, :], in_=qkv[:, 0, sl])
                nc.vector.bn_stats(out=qstats[:, 1, si, :], in_=qkv[:, 1, sl])

            # ---- Phase F: per 128-n chunk: k_T, q_T, gram, kss
            for ic in range(N128):
                c0 = ic * 128
                gidx = s * N128 + ic
                kps = pspool.tile([128, 192], F32, tag="ps", name=f"kps{s}_{ic}")
                for j, (src, pb, base, sz) in enumerate(KCC):
                    data = (qkv if src == "qkv" else dw)
                    lhsT = data[base:base + sz, pb, c0:c0 + 128]
                    rhs = nkw_s[base:base + sz, j, :]
                    nc.tensor.matmul(kps[:, :], lhsT, rhs, start=(j == 0), stop=(j == 3),
                                     tile_position=(base, 0) if base else None)
                kT = smpool.tile([128, 192], BF16, tag="kT")
                nc.vector.scalar_tensor_tensor(
                    out=kT[:], in0=kps[:], scalar=1.0, in1=nkb_bc[:],
                    op0=mybir.AluOpType.mult, op1=mybir.AluOpType.add)
                qps = pspool.tile([128, 256], BF16, tag="ps", name=f"qps{s}_{ic}")
                nc.tensor.transpose(qps[:, 0:128], qkv[:, 0, c0:c0 + 128], id_s[:, :])
                nc.tensor.transpose(qps[:, 128:256], qkv[:, 1, c0:c0 + 128], id_s[:, :])
                qT = smpool.tile([128, 256], BF16, tag="qT")
                nc.scalar.copy(out=qT[:], in_=qps[:])
                nc.tensor.matmul(gramA[:, :], kT[:, 0:128], qT[:],
                                 start=(gidx == 0), stop=(gidx == n128_total - 1))
                nc.tensor.matmul(gramB[:, :], kT[:, 128:192], qT[:],
                                 start=(gidx == 0), stop=(gidx == n128_total - 1))
                ksq = smpool.tile([128, 192], BF16, tag="ksq")
                nc.vector.tensor_mul(ksq[:], kT[:], kT[:])
                nc.tensor.matmul(kss_ps[:, :], ones_s[:, :], ksq[:],
                                 start=(gidx == 0), stop=(gidx == n128_total - 1))

        qmv = statpool.tile([128, 2, 2], F32)
        vmv = statpool.tile([128, 2, 2], F32)
        nc.vector.memset(qmv[:], 0.0)
        nc.vector.memset(vmv[:], 0.0)
        nc.vector.bn_aggr(out=qmv[:, 0, :], in_=qstats[:, 0, :, :])
        nc.vector.bn_aggr(out=qmv[:, 1, :], in_=qstats[:, 1, :, :])
        nc.vector.bn_aggr(out=vmv[:, 0, :], in_=vstats[:, 0, :, :])
        nc.vector.bn_aggr(out=vmv[:64, 1, :], in_=vstats[:64, 1, :, :])
        nc.sync.dma_start(out=qstats_out[:], in_=qmv[:])
        nc.sync.dma_start(out=vstats_out[:], in_=vmv[:])
        gA = statpool.tile([128, 256], F32)
        gB = statpool.tile([64, 256], F32)
        kssb = statpool.tile([1, 192], F32)
        nc.scalar.copy(out=gA[:], in_=gramA[:])
        nc.scalar.copy(out=gB[:], in_=gramB[:])
        nc.scalar.copy(out=kssb[:], in_=kss_ps[:])
        nc.sync.dma_start(out=gram_out[0:128, :], in_=gA[:])
        nc.sync.dma_start(out=gram_out[128:192, :], in_=gB[:])
        nc.sync.dma_start(out=kss_out[:], in_=kssb[:])

    nc.compile()
    return nc


def build_l2(R=64, W=128):
    NS = R * W
    nc = bacc.Bacc("TRN2", target_bir_lowering=False, debug=False, num_devices=8)
    v_in = nc.dram_tensor("v_in", [192, NS], BF16, kind="ExternalInput").ap()
    awT = nc.dram_tensor("awT", [128, 2, 192], BF16, kind="ExternalInput").ap()
    pbias = nc.dram_tensor("pbias", [128, 2], F32, kind="ExternalInput").ap()
    out = nc.dram_tensor("out", [192, NS], F32, kind="ExternalOutput").ap()

    with tile.TileContext(nc) as tc, ExitStack() as ctx:
        wpool = ctx.enter_context(tc.tile_pool(name="w", bufs=1))
        vpool = ctx.enter_context(tc.tile_pool(name="v", bufs=1))
        opool = ctx.enter_context(tc.tile_pool(name="o", bufs=4))
        pspool = ctx.enter_context(tc.tile_pool(name="ps", bufs=4, space="PSUM"))

        aw = wpool.tile([128, 2, 192], BF16)
        nc.sync.dma_start(out=aw[:], in_=awT[:])
        pb = wpool.tile([128, 2], F32)
        nc.sync.dma_start(out=pb[:], in_=pbias[:])
        vt = vpool.tile([128, 2, NS], BF16)
        nc.sync.dma_start(out=vt[:, 0, :], in_=v_in[0:128, :])
        nc.sync.dma_start(out=vt[:64, 1, :], in_=v_in[128:192, :])

        for c0 in range(0, NS, 512):
            for mb in range(2):
                msz = 128 if mb == 0 else 64
                ps = pspool.tile([128, 512], F32, tag="ps")
                nc.tensor.matmul(ps[:msz, :], aw[:, 0, mb * 128:mb * 128 + msz],
                                 vt[:, 0, c0:c0 + 512], start=True, stop=False)
                nc.tensor.matmul(ps[:msz, :], aw[:64, 1, mb * 128:mb * 128 + msz],
                                 vt[:64, 1, c0:c0 + 512], start=False, stop=True)
                ot = opool.tile([128, 512], F32, tag="ot")
                nc.scalar.activation(out=ot[:msz, :], in_=ps[:msz, :],
                                     func=mybir.ActivationFunctionType.Identity,
                                     bias=pb[:msz, mb:mb + 1], scale=1.0)
                nc.sync.dma_start(out=out[mb * 128:mb * 128 + msz, c0:c0 + 512],
                                  in_=ot[:msz, :])
    nc.compile()
    return nc



WP = 130


def _dw_channel_at(pb, p):
    """kv channel (0..383) stored at partition p of dw pblock pb."""
    for (ipb, ib, opb, ob) in DWBLK:
        if opb == pb and ob <= p < ob + 64:
            return 128 * ipb + ib + (p - ob)
    return None


def prep_weights(w):
    """w: dict of reference weights (numpy f32). Returns dict of L1 input arrays."""
    out = {}
    qw = w["q_w"][:, :, 0, 0]          # (576, 192)
    qwT = np.zeros((128, 2, 768), np.float32)
    for (pb, h, x1b, qb, ch0, nch) in qkv_halves():
        win = 128 * pb + 64 * h
        qwT[0:128, 0, win:win + nch] = qw.T[0:128, ch0:ch0 + nch]
        qwT[0:64, 1, win:win + nch] = qw.T[128:192, ch0:ch0 + nch]
    out["qwT"] = qwT.astype(bf16)

    qdw = w["qdw_w"]                   # (576, 3, 3, 3) out, in-per-group, ky, kx
    qdwT = np.zeros((128, 54, 128), np.float32)
    for dy in (-1, 0, 1):
        for dx in (-1, 0, 1):
            ti = 3 * dy + dx + 4
            for (pb, h, x1b, qb, ch0, nch) in qkv_halves():
                for gl in range(nch // 3):
                    for i in range(3):
                        for j in range(3):
                            qdwT[x1b + 3 * gl + i, 6 * ti + pb, qb + 3 * gl + j] = \
                                qdw[ch0 + 3 * gl + j, i, dy + 1, dx + 1]
    out["qdw_wT"] = qdwT.astype(bf16)

    kvw = w["kv_w"]                    # (384, 192, 3, 3)
    kvT = np.zeros((128, 2, 9, 384), np.float32)
    for dy in (-1, 0, 1):
        for dx in (-1, 0, 1):
            ti = 3 * dy + dx + 4
            t = kvw[:, :, dy + 1, dx + 1].T   # (192, 384)
            kvT[:, 0, ti, :] = t[0:128]
            kvT[0:64, 1, ti, :] = t[128:192]
    out["kv_wT"] = kvT.astype(bf16)
    kvp = np.zeros((128, 4, 384), np.float32)
    for j, dx in enumerate((-1, 0, 1)):
        kvp[0:64, j, :] = kvw[:, :, 0, dx + 1].T[128:192]    # (-1, dx)
        kvp[64:128, j, :] = kvw[:, :, 1, dx + 1].T[128:192]  # (0, dx)
    kvp[0:64, 3, :] = kvw[:, :, 2, 0].T[128:192]     # (1, -1)
    kvp[64:128, 3, :] = kvw[:, :, 2, 2].T[128:192]   # (1, +1)
    out["kvp_wT"] = kvp.astype(bf16)

    kvdw = w["kvdw_w"][:, 0]           # (384, 3, 3)
    dwT = np.zeros((128, 54, 64), np.float32)
    for dy in (-1, 0, 1):
        for dx in (-1, 0, 1):
            ti = 3 * dy + dx + 4
            d = kvdw[:, dy + 1, dx + 1]
            for blk, (ipb, ib, opb, ob) in enumerate(DWBLK):
                ch0 = 128 * ipb + ib
                dwT[ib:ib + 64, 6 * ti + blk, :] = np.diag(d[ch0:ch0 + 64])
    out["kvdw_wT"] = dwT.astype(bf16)

    nk = w["newk_w"][:, :, 0, 0]       # (192, 384): in = [k(192) | k_mask(192)]
    nkm = np.zeros((128, 5, 192), np.float32)
    for j, (src, pb, base, sz) in enumerate(KCC):
        for p in range(base, base + sz):
            if src == "qkv":
                ch = qkv_channel_at(pb, p)
                if ch is not None:
                    nkm[p, j, :] = nk[:, ch - 192]      # k part: qkv ch 192-383
            else:
                ch = _dw_channel_at(pb, p)
                if ch is not None and ch < 192:
                    nkm[p, j, :] = nk[:, 192 + ch]      # k_mask: dw ch 0-191
    out["newk_w_m"] = nkm.astype(bf16)

    nv = w["newv_w"][:, :, 0, 0]       # (192, 384): in = [v(192) | v_mask(192)]
    nvT = np.zeros((128, 5, 192), np.float32)
    for j, (src, pb, base, sz) in enumerate(VCC):
        for p in range(base, base + sz):
            if src == "qkv":
                ch = qkv_channel_at(pb, p)
                if ch is not None:
                    nvT[p, j, :] = nv[:, ch - 384]      # v part: qkv ch 384-575
            else:
                ch = _dw_channel_at(pb, p)
                if ch is not None and ch >= 192:
                    nvT[p, j, :] = nv[:, ch]            # v_mask: dw ch 192-383
    out["newv_wT"] = nvT.astype(bf16)

    out["ident"] = np.eye(128, dtype=bf16)
    out["ones_col"] = np.ones((128, 1), dtype=bf16)

    x1b = np.zeros((128, 6), np.float32)
    qkvb = np.zeros((128, 6), np.float32)
    for (pb, h, x1b_base, qb, ch0, nch) in qkv_halves():
        x1b[x1b_base:x1b_base + nch, pb] = w["q_b"][ch0:ch0 + nch]
        qkvb[qb:qb + nch, pb] = w["qdw_b"][ch0:ch0 + nch]
    out["x1_bias"] = x1b
    out["qkv_bias"] = qkvb

    kvb = np.zeros((128, 3), np.float32)
    kvb[:, 0] = w["kv_b"][0:128]
    kvb[0:64, 1] = w["kv_b"][128:192]
    kvb[64:128, 1] = w["kv_b"][192:256]
    kvb[:, 2] = w["kv_b"][256:384]
    out["kv_bias"] = kvb
    dwb = np.zeros((128, 3), np.float32)
    for (ipb, ib, opb, ob) in DWBLK:
        dwb[ob:ob + 64, opb] = w["kvdw_b"][128 * ipb + ib:128 * ipb + ib + 64]
    out["kvdw_bias"] = dwb
    out["newk_b_row"] = w["newk_b"][None, :].astype(np.float32)
    nvb = np.zeros((128, 2), np.float32)
    nvb[:, 0] = w["newv_b"][0:128]
    nvb[0:64, 1] = w["newv_b"][128:192]
    out["newv_bias"] = nvb
    return out


def prep_masks(R, H, half):
    m = np.zeros((R + 2, WP), np.float32)
    for r in range(R + 2):
        g = half * R + (r - 1)
        if 0 <= g < H:
            m[r, 1:129] = 1.0
    return m.reshape(1, -1)


def prep_core(x, xm, b, half, R, H):
    xp = np.zeros((192, R + 2, WP), np.float32)
    mp = np.zeros((192, R + 4, WP), np.float32)
    for r in range(R + 2):
        g = half * R + (r - 1)
        if 0 <= g < H:
            xp[:, r, 1:129] = x[b, :, g, :]
    for r in range(R + 4):
        g = half * R + (r - 2)
        if 0 <= g < H:
            mp[:, r, 1:129] = xm[b, :, g, :]
    xp = xp.reshape(192, -1).astype(bf16)
    mp = mp.reshape(192, -1)
    L = mp.shape[1]
    hi = mp[128:192]
    d1 = np.zeros((128, L), np.float32)
    d2 = np.zeros((128, L), np.float32)
    d1[0:64] = hi
    d1[64:128, :L - 130] = hi[:, 130:]
    d2[0:64] = hi
    d2[64:128, :L - 2] = hi[:, 2:]
    return {
        "x_lo": xp[0:128], "x_hi": xp[128:192],
        "xm_lo": mp[0:128].astype(bf16),
        "xm_d1": d1.astype(bf16), "xm_d2": d2.astype(bf16),
        "mask_rc": prep_masks(R, H, half).astype(bf16),
    }


def _q_maps():
    """q channel c (0..191) -> (pblock 0/1, partition)."""
    part = np.zeros(192, np.int64)
    pblk = np.zeros(192, np.int64)
    for (pb, h, x1b, qb, ch0, nch) in qkv_halves():
        if pb >= 2:
            continue
        for i in range(nch):
            pblk[ch0 + i] = pb
            part[ch0 + i] = qb + i
    return pblk, part


def _ss_from_qstats(stats, n_half):
    pblk, part = _q_maps()
    mv = stats.astype(np.float64)
    return (mv[part, pblk, 1] + mv[part, pblk, 0] ** 2) * n_half


def _ss_from_vstats(stats, n_half):
    ss = np.zeros(192, np.float64)
    mv = stats.astype(np.float64)
    ss[0:128] = (mv[0:128, 0, 1] + mv[0:128, 0, 0] ** 2) * n_half
    ss[128:192] = (mv[0:64, 1, 1] + mv[0:64, 1, 0] ** 2) * n_half
    return ss


def glue(res0, res1, temperature, proj_w, proj_b, n_half):
    """Combine two half-core L1 results -> L2 inputs (awT, pbias)."""
    G = res0["gram_out"].astype(np.float64) + res1["gram_out"].astype(np.float64)
    pblk, part = _q_maps()
    qcol = pblk * 128 + part
    G = G[:, qcol]                              # (d, c): sum_n k[d,n] q[c,n]
    qss = _ss_from_qstats(res0["qstats_out"], n_half) + _ss_from_qstats(res1["qstats_out"], n_half)
    vss = _ss_from_vstats(res0["vstats_out"], n_half) + _ss_from_vstats(res1["vstats_out"], n_half)
    kss = (res0["kss_out"].astype(np.float64) + res1["kss_out"].astype(np.float64))[0]
    qn = np.maximum(np.sqrt(qss), 1e-12)
    kn = np.maximum(np.sqrt(kss), 1e-12)
    vn = np.maximum(np.sqrt(vss), 1e-12)
    A = G.T / (qn[:, None] * kn[None, :])      # (c, d)
    M = np.zeros((192, 192), np.float64)
    t = np.asarray(temperature).reshape(-1)
    for h in range(8):
        sl = slice(24 * h, 24 * h + 24)
        a = A[sl, sl] * t[h]
        a = a - a.max(axis=-1, keepdims=True)
        e = np.exp(a)
        sm = e / e.sum(axis=-1, keepdims=True)
        M[sl, sl] = sm / vn[None, sl]
    At = proj_w[:, :, 0, 0].astype(np.float64) @ M   # (out-ch o, d)
    awT = np.zeros((128, 2, 192), np.float32)
    awT[:, 0, :] = At.T[0:128]
    awT[0:64, 1, :] = At.T[128:192]
    pbias = np.zeros((128, 2), np.float32)
    pbias[:, 0] = proj_b[0:128]
    pbias[0:64, 1] = proj_b[128:192]
    return {"awT": awT.astype(bf16), "pbias": pbias}




# ---------------- driver: kernel(**inputs) ----------------
from concourse.bass_utils import run_bass_kernel_spmd

R_FULL, H_FULL, B_FULL = 64, 128, 4
_NC1 = None
_NC2 = None


def _get_progs():
    global _NC1, _NC2
    if _NC1 is None:
        _NC1 = build_l1(R=R_FULL, S=16)
        _NC2 = build_l2(R=R_FULL)
    return _NC1, _NC2


def kernel(**inputs):
    inputs = {k: np.asarray(v) for k, v in inputs.items()}
    x, xm = inputs["x"], inputs["x_mask"]
    nc1, nc2 = _get_progs()
    wprep = prep_weights(inputs)
    in_maps = []
    for core in range(8):
        b, half = core // 2, core % 2
        m = dict(wprep)
        m.update(prep_core(x, xm, b, half, R_FULL, H_FULL))
        in_maps.append(m)
    res1 = run_bass_kernel_spmd(nc1, in_maps, list(range(8))).results

    n_half = R_FULL * 128
    in_maps2 = []
    for core in range(8):
        b, half = core // 2, core % 2
        if half == 0:
            l2c = glue(res1[2 * b], res1[2 * b + 1], inputs["temperature"],
                       inputs["proj_w"], inputs["proj_b"], n_half)
        m = dict(l2c)
        m["v_in"] = res1[core]["v_out"]
        in_maps2.append(m)
    res2 = run_bass_kernel_spmd(nc2, in_maps2, list(range(8))).results

    out = np.empty((B_FULL, 192, H_FULL, 128), np.float32)
    for core in range(8):
        b, half = core // 2, core % 2
        out[b, :, half * R_FULL:(half + 1) * R_FULL, :] = \
            res2[core]["out"].reshape(192, R_FULL, 128)
    return out



# revision 7
# speedup vs baseline: 1.5520x; 1.5520x over previous
"""Trainium2 Bass kernel for nn_Cross_Attention (sparse_attention, 8 cores).

fp8(e4m3)+DoubleRow version: phases A/C/E/F and L2 run fp8 DoubleRow matmuls
(two 128-deep contraction chunks per pass at ~0.57 cyc/col); B/D keep bf16
quadrant matmuls but emit fp8. Power-of-2 scale bookkeeping throughout.
Shards 4 samples x 2 row-halves across 8 NeuronCores, glues partials on host.
"""
import sys
sys.path.insert(0, "/opt/trn_rl_repo")
import numpy as np
import ml_dtypes

import concourse.bass as bass
import concourse.tile as tile
from concourse import bacc, mybir
from contextlib import ExitStack


BF16 = mybir.dt.bfloat16
F8 = mybir.dt.float8e4
F32 = mybir.dt.float32
bf16 = ml_dtypes.bfloat16
e4m3 = ml_dtypes.float8_e4m3
DR = mybir.MatmulPerfMode.DoubleRow

TAPS = [(dy, dx) for dy in (-1, 0, 1) for dx in (-1, 0, 1)]

# power-of-2 scales
SXI = 16.0        # x, x_mask fp8 pre-scale (2^4)
SW = 1024.0       # conv-weight fp8 pre-scale (2^10)
MINV = 1.0 / (SXI * SW)   # mask value de-scaling x1/kv1 back to true (2^-14)
SQKV = 512.0      # qkv fp8 storage scale (2^9)
SDW8 = 256.0      # dw fp8 storage scale (2^8)
SK8 = 512.0       # kT fp8 storage scale (2^9)
SV8 = 512.0       # v fp8 storage scale (2^9)
SKE = 2.0 ** 19   # newk/newv psum scale


def qf8(a, s):
    return np.clip(np.asarray(a, np.float32) * s, -240, 240).astype(e4m3)


def qkv_halves():
    """Per (pb, half): (x1_base, qkv_base, ch0, nch).  ch0 = qkv-global channel."""
    out = []
    for pb in range(6):
        P, odd = pb // 2, pb % 2
        for h in (0, 1):
            nch = 3 if (odd and h == 1) else 63
            ch0 = 3 * (64 * P + 42 * odd + 21 * h)
            x1b = 64 * h
            qb = 64 * h if not odd else 64 * (1 - h)
            out.append((pb, h, x1b, qb, ch0, nch))
    return out


def qkv_channel_at(pb, p):
    """qkv-global channel stored at partition p of qkv pblock pb, or None."""
    for (pb2, h, x1b, qb, ch0, nch) in qkv_halves():
        if pb2 == pb and qb <= p < qb + nch:
            return ch0 + (p - qb)
    return None


# newk/newv input chunk pairs (DoubleRow planes), see prep_weights
# kvdw 64-blocks: (in_pb, in_base, out_pb, out_base); dw pb1 halves swapped
DWBLK = [(0, 0, 0, 0), (0, 64, 0, 64), (1, 0, 1, 64),
         (1, 64, 1, 0), (2, 0, 2, 0), (2, 64, 2, 64)]


def _bcast(ap, p):
    return bass.AP(tensor=ap.tensor, offset=ap.offset, ap=[[0, p]] + list(ap.ap[1:]))


def _dr3(tile_ap, base, delta, cs):
    """[P, 2, cs] DoubleRow rhs view of a tile at element offset base,
    plane delta `delta`."""
    return bass.AP(tensor=tile_ap.tensor, offset=tile_ap.offset + base,
                   ap=[list(tile_ap.ap[0]), [delta, 2], [1, cs]])


def build_l1(R=64, W=128, S=16):
    assert W == 128 and R % S == 0 and S % 4 == 0
    WP = W + 2  # 130
    NSLAB = R // S
    NS = S * W
    XCOLS = (S + 2) * WP
    MCOLS = (S + 4) * WP
    MCW = MCOLS + 2
    N128 = NS // 128

    nc = bacc.Bacc("TRN2", target_bir_lowering=False, debug=False, num_devices=8)

    def din(name, shape, dt=F8):
        return nc.dram_tensor(name, shape, dt, kind="ExternalInput").ap()

    def dout(name, shape, dt=F32):
        return nc.dram_tensor(name, shape, dt, kind="ExternalOutput").ap()

    xc = din("xc", [128, 2, (R + 2) * WP])
    xm_lo = din("xm_lo", [128, (R + 4) * WP])
    xm_d1 = din("xm_d1", [128, (R + 4) * WP])
    xm_d2 = din("xm_d2", [128, (R + 4) * WP])
    qwT = din("qwT", [128, 2, 768])            # 12 zero-padded 64-ch windows
    qdw_wT = din("qdw_wT", [128, 9 * 6, 128], BF16)
    cw = din("cw", [128, 7, 2, 384])           # kv 3x3 tap-pair weights
    kvdw_wT = din("kvdw_wT", [128, 9 * 6, 64], BF16)
    nkw = din("nkw", [128, 2, 2, 192])
    nvw = din("nvw", [128, 2, 2, 192])
    ident = din("ident", [128, 128])
    ones_col = din("ones_col", [128, 1], BF16)
    x1_bias = din("x1_bias", [128, 6], F32)
    qkv_bias = din("qkv_bias", [128, 6], F32)
    kv_bias = din("kv_bias", [128, 3], F32)
    kvdw_bias = din("kvdw_bias", [128, 3], F32)
    newk_b_row = din("newk_b_row", [1, 192], F32)
    newv_bias = din("newv_bias", [128, 2], F32)
    mask_rc = din("mask_rc", [1, (R + 2) * WP], BF16)

    v_out = dout("v_out", [128, 2, R * W], F8)
    gram_out = dout("gram_out", [192, 256])
    kss_out = dout("kss_out", [1, 192])
    qstats_out = dout("qstats_out", [128, 2, 2])
    vstats_out = dout("vstats_out", [128, 2, 2])

    with tile.TileContext(nc) as tc, ExitStack() as ctx:
        wpool = ctx.enter_context(tc.tile_pool(name="weights", bufs=1))
        xpool = ctx.enter_context(tc.tile_pool(name="xslab", bufs=2))
        bigpool = ctx.enter_context(tc.tile_pool(name="big", bufs=1))
        midpool = ctx.enter_context(tc.tile_pool(name="mid", bufs=2))
        smpool = ctx.enter_context(tc.tile_pool(name="small", bufs=4))
        statpool = ctx.enter_context(tc.tile_pool(name="stats", bufs=1))
        pspool = ctx.enter_context(tc.tile_pool(name="ps", bufs=5, space="PSUM"))
        pspers = ctx.enter_context(tc.tile_pool(name="pspers", bufs=1, space="PSUM"))

        def load1(ap_in, shape, dt=F8, eng=None):
            t = wpool.tile(shape, dt, tag=ap_in.tensor.name)
            (eng or nc.sync).dma_start(out=t[:ap_in.shape[0]], in_=ap_in[:])
            return t

        # phase-A-critical constants on the sync queue; the rest via gpsimd queue
        qwT_s = load1(qwT, [128, 2, 768])
        x1b_s = load1(x1_bias, [128, 6], F32)
        g = nc.gpsimd
        qdw_s = load1(qdw_wT, [128, 9 * 6, 128], BF16, eng=g)
        cw_s = load1(cw, [128, 7, 2, 384], eng=g)
        kvdw_s = load1(kvdw_wT, [128, 9 * 6, 64], BF16, eng=g)
        nkw_s = load1(nkw, [128, 2, 2, 192], eng=g)
        nvw_s = load1(nvw, [128, 2, 2, 192], eng=g)
        id_s = load1(ident, [128, 128], eng=g)
        ones_s = load1(ones_col, [128, 1], BF16, eng=g)
        qkvb_s = load1(qkv_bias, [128, 6], F32, eng=g)
        kvb_s = load1(kv_bias, [128, 3], F32, eng=g)
        dwb_s = load1(kvdw_bias, [128, 3], F32, eng=g)
        nvb_s = load1(newv_bias, [128, 2], F32, eng=g)
        nkb_bc = wpool.tile([128, 192], F32, tag="nkb_bc")
        nc.gpsimd.dma_start(out=nkb_bc[:], in_=_bcast(newk_b_row[0:1, :], 128))

        gramA = pspers.tile([128, 256], F32)
        gramB = pspers.tile([64, 256], F32)
        kss_ps = pspers.tile([1, 192], F32)

        qstats = statpool.tile([128, 2, NSLAB * (NS // 512), 6], F32)
        vstats = statpool.tile([128, 2, NSLAB * (NS // 512), 6], F32)

        n128_total = NSLAB * N128

        for s in range(NSLAB):
            xsl = xpool.tile([128, 2, XCOLS], F8, tag="xsl")
            mc = xpool.tile([128, 3, MCW], F8, tag="mc")
            off = s * S * WP
            nc.sync.dma_start(out=xsl[:], in_=xc[:, :, off:off + XCOLS])
            nc.sync.dma_start(out=mc[:, 0, 1:1 + MCOLS], in_=xm_lo[:, off:off + MCOLS])
            nc.sync.dma_start(out=mc[:, 1, 1:1 + MCOLS], in_=xm_d1[:, off:off + MCOLS])
            nc.sync.dma_start(out=mc[:, 2, 1:1 + MCOLS], in_=xm_d2[:, off:off + MCOLS])
            for pl in range(3):
                nc.vector.memset(mc[:, pl, 0:1], 0.0)
                nc.vector.memset(mc[:, pl, MCW - 1:MCW], 0.0)
            mtile = xpool.tile([128, XCOLS], BF16, tag="mtile")
            nc.sync.dma_start(out=mtile[:], in_=_bcast(mask_rc[0:1, off:off + XCOLS], 128))

            # ---- Phase A: x1 = 1x1(x): one fp8 DoubleRow pass per (pb, c0)
            x1 = bigpool.tile([128, 6, XCOLS], BF16, tag="x1")
            for pb in range(6):
                for c0 in range(0, XCOLS, 512):
                    cs = min(512, XCOLS - c0)
                    ps = pspool.tile([128, 512], F32, tag="ps", name=f"psA{s}_{pb}_{c0}")
                    nc.tensor.matmul(
                        ps[:, :cs], qwT_s[:, :, 128 * pb:128 * pb + 128],
                        xsl[:, :, c0:c0 + cs], start=True, stop=True, perf_mode=DR)
                    nc.vector.scalar_tensor_tensor(
                        out=x1[:, pb, c0:c0 + cs], in0=ps[:, :cs],
                        scalar=x1b_s[:, pb:pb + 1], in1=mtile[:, c0:c0 + cs],
                        op0=mybir.AluOpType.add, op1=mybir.AluOpType.mult)

            # ---- Phase C: kv1 = 3x3(xm): 7 fp8 DoubleRow tap-pair passes
            # pair rhs (base offset into mc, plane delta):
            CPASS = [(0, 1), (2, WP - 2), (WP + 1, 1), (2 * WP, 1),
                     (2 * WP + 2, MCW - 1),
                     (MCW, 1), (MCW + 2, MCW + 2 * WP - 2)]
            kv1 = bigpool.tile([128, 3, XCOLS], BF16, tag="kv1")
            for c0 in range(0, XCOLS, 512):
                cs = min(512, XCOLS - c0)
                for pb in range(3):
                    ps = pspool.tile([128, 512], F32, tag="ps", name=f"psC{s}_{pb}_{c0}")
                    for i, (base, delta) in enumerate(CPASS):
                        nc.tensor.matmul(
                            ps[:, :cs], cw_s[:, i, :, 128 * pb:128 * pb + 128],
                            _dr3(mc[:, 0, :], base + c0, delta, cs),
                            start=(i == 0), stop=(i == 6), perf_mode=DR)
                    nc.vector.scalar_tensor_tensor(
                        out=kv1[:, pb, c0:c0 + cs], in0=ps[:, :cs],
                        scalar=kvb_s[:, pb:pb + 1], in1=mtile[:, c0:c0 + cs],
                        op0=mybir.AluOpType.add, op1=mybir.AluOpType.mult)

            # ---- Phase B: qkv = qdw(x1): pblock pairs, 4 concurrent 64x64 blocks
            qkv = bigpool.tile([128, 6, NS], F8, tag="qkv")
            x1v = [x1[:, pb, :].rearrange("p (r c) -> p r c", c=WP) for pb in range(6)]
            for sset in range(3):
                for ic in range(S // 4):
                    r0 = ic * 4
                    pse = pspool.tile([128, 512], F32, tag="ps", name=f"psBe{s}_{sset}_{ic}")
                    pso = pspool.tile([128, 512], F32, tag="ps", name=f"psBo{s}_{sset}_{ic}")
                    for ti, (dy, dx) in enumerate(TAPS):
                        for j in (0, 1):
                            pb = 2 * sset + j
                            pst = pse if j == 0 else pso
                            for h in (0, 1):
                                qb = 64 * h if j == 0 else 64 * (1 - h)
                                rhs = x1v[pb][64 * h:64 * h + 64,
                                              r0 + 1 + dy:r0 + 5 + dy, 1 + dx:1 + dx + 128]
                                nc.tensor.matmul(
                                    pst[qb:qb + 64, :],
                                    qdw_s[64 * h:64 * h + 64, 6 * ti + pb, qb:qb + 64],
                                    rhs, start=(ti == 0), stop=(ti == 8),
                                    tile_position=(64 * h, qb), skip_group_check=True)
                    for j in (0, 1):
                        pb = 2 * sset + j
                        nc.scalar.activation(out=qkv[:, pb, r0 * 128:(r0 + 4) * 128],
                                             in_=(pse if j == 0 else pso)[:, :],
                                             func=mybir.ActivationFunctionType.Identity,
                                             bias=qkvb_s[:, pb:pb + 1], scale=SQKV)

            # ---- Phase D: kvdw via 6 concurrent 64x64 diag blocks
            dw = bigpool.tile([128, 3, NS], F8, tag="dw")
            kv1v = [kv1[:, pb, :].rearrange("p (r c) -> p r c", c=WP) for pb in range(3)]
            for ic in range(S // 4):
                r0 = ic * 4
                pss = [pspool.tile([128, 512], F32, tag="ps", name=f"psD{s}_{ic}_{i}")
                       for i in range(3)]
                for ti, (dy, dx) in enumerate(TAPS):
                    for blk, (ipb, ib, opb, ob) in enumerate(DWBLK):
                        rhs = kv1v[ipb][ib:ib + 64,
                                        r0 + 1 + dy:r0 + 5 + dy, 1 + dx:1 + dx + 128]
                        nc.tensor.matmul(pss[opb][ob:ob + 64, :],
                                         kvdw_s[ib:ib + 64, 6 * ti + blk, :],
                                         rhs, start=(ti == 0), stop=(ti == 8),
                                         tile_position=(ib, ob), skip_group_check=True)
                for pb in range(3):
                    nc.scalar.activation(out=dw[:, pb, r0 * 128:(r0 + 4) * 128],
                                         in_=pss[pb][:, :],
                                         func=mybir.ActivationFunctionType.Identity,
                                         bias=dwb_s[:, pb:pb + 1], scale=SDW8)

            # ---- Phase E: v = newv(v_cc): 2 fp8 DoubleRow passes per (c0, mb)
            vt = midpool.tile([128, 2, NS], F8, tag="vt")
            for ic in range(NS // 512):
                c0 = ic * 512
                for mb in range(2):
                    msz = 128 if mb == 0 else 64
                    ps = pspool.tile([128, 512], F32, tag="ps", name=f"psE{s}_{ic}_{mb}")
                    nc.tensor.matmul(ps[:msz, :], nvw_s[:, 0, :, mb * 128:mb * 128 + msz],
                                     qkv[:, 4:6, c0:c0 + 512], start=True, stop=False,
                                     perf_mode=DR)
                    nc.tensor.matmul(ps[:msz, :], nvw_s[:, 1, :, mb * 128:mb * 128 + msz],
                                     dw[:, 1:3, c0:c0 + 512], start=False, stop=True,
                                     perf_mode=DR)
                    nc.scalar.activation(out=vt[:msz, mb, c0:c0 + 512], in_=ps[:msz, :],
                                         func=mybir.ActivationFunctionType.Identity,
                                         bias=nvb_s[:msz, mb:mb + 1], scale=SV8 / SKE)
            nc.sync.dma_start(out=v_out[:, 0, s * NS:(s + 1) * NS], in_=vt[:, 0, :])
            nc.sync.dma_start(out=v_out[0:64, 1, s * NS:(s + 1) * NS], in_=vt[:64, 1, :])
            for sub in range(NS // 512):
                si = s * (NS // 512) + sub
                sl = slice(sub * 512, (sub + 1) * 512)
                nc.vector.bn_stats(out=vstats[:, 0, si, :], in_=vt[:, 0, sl])
                nc.vector.bn_stats(out=vstats[:64, 1, si, :], in_=vt[:64, 1, sl])
                nc.vector.bn_stats(out=qstats[:, 0, si, :], in_=qkv[:, 0, sl])
                nc.vector.bn_stats(out=qstats[:, 1, si, :], in_=qkv[:, 1, sl])

            # ---- Phase F: per 256-px group: kT pair, qT pair, DoubleRow gram, kss
            for gi in range(N128 // 2):
                gidx = s * (N128 // 2) + gi
                kT2 = smpool.tile([128, 2, 192], F8, tag="kT2")
                # fp8 transpose requires output element step 2: interleaved psum
                qps = pspool.tile([128, 2, 512], F8, tag="ps", name=f"qps{s}_{gi}")
                qps_f = qps[:, 0, :]
                pstr = list(qps_f.ap[0])
                for ci in (0, 1):
                    c0 = (2 * gi + ci) * 128
                    kps = pspool.tile([128, 192], F32, tag="ps", name=f"kps{s}_{gi}_{ci}")
                    nc.tensor.matmul(kps[:, :], qkv[:, 2:4, c0:c0 + 128],
                                     nkw_s[:, 0, :, :], start=True, stop=False,
                                     perf_mode=DR)
                    nc.tensor.matmul(kps[:, :], dw[:, 0:2, c0:c0 + 128],
                                     nkw_s[:, 1, :, :], start=False, stop=True,
                                     perf_mode=DR)
                    nc.vector.scalar_tensor_tensor(
                        out=kT2[:, ci, :], in0=kps[:], scalar=SK8 / SKE, in1=nkb_bc[:],
                        op0=mybir.AluOpType.mult, op1=mybir.AluOpType.add)
                    for ch in (0, 1):
                        o2 = bass.AP(tensor=qps_f.tensor,
                                     offset=qps_f.offset + ci * 512 + ch * 256,
                                     ap=[pstr, [2, 128]])
                        nc.tensor.transpose(o2, qkv[:, ch, c0:c0 + 128], id_s[:, :])
                    ksq = smpool.tile([128, 192], BF16, tag="ksq")
                    nc.vector.tensor_mul(ksq[:], kT2[:, ci, :], kT2[:, ci, :])
                    nc.tensor.matmul(kss_ps[:, :], ones_s[:, :], ksq[:],
                                     start=(gidx == 0 and ci == 0),
                                     stop=(gidx == n128_total // 2 - 1 and ci == 1))
                qT2 = smpool.tile([128, 2, 256], F8, tag="qT2")
                qps_v = bass.AP(tensor=qps_f.tensor, offset=qps_f.offset,
                                ap=[pstr, [512, 2], [256, 2], [2, 128]])
                qT2_v = qT2[:].rearrange("p c (h n) -> p c h n", h=2)
                nc.scalar.copy(out=qT2_v, in_=qps_v)
                nc.tensor.matmul(gramA[:, :], kT2[:, :, 0:128], qT2[:],
                                 start=(gidx == 0), stop=(gidx == n128_total // 2 - 1),
                                 perf_mode=DR)
                nc.tensor.matmul(gramB[:, :], kT2[:, :, 128:192], qT2[:],
                                 start=(gidx == 0), stop=(gidx == n128_total // 2 - 1),
                                 perf_mode=DR)

        qmv = statpool.tile([128, 2, 2], F32)
        vmv = statpool.tile([128, 2, 2], F32)
        nc.vector.memset(qmv[:], 0.0)
        nc.vector.memset(vmv[:], 0.0)
        nc.vector.bn_aggr(out=qmv[:, 0, :], in_=qstats[:, 0, :, :])
        nc.vector.bn_aggr(out=qmv[:, 1, :], in_=qstats[:, 1, :, :])
        nc.vector.bn_aggr(out=vmv[:, 0, :], in_=vstats[:, 0, :, :])
        nc.vector.bn_aggr(out=vmv[:64, 1, :], in_=vstats[:64, 1, :, :])
        nc.sync.dma_start(out=qstats_out[:], in_=qmv[:])
        nc.sync.dma_start(out=vstats_out[:], in_=vmv[:])
        gA = statpool.tile([128, 256], F32)
        gB = statpool.tile([64, 256], F32)
        kssb = statpool.tile([1, 192], F32)
        nc.scalar.copy(out=gA[:], in_=gramA[:])
        nc.scalar.copy(out=gB[:], in_=gramB[:])
        nc.scalar.copy(out=kssb[:], in_=kss_ps[:])
        nc.sync.dma_start(out=gram_out[0:128, :], in_=gA[:])
        nc.sync.dma_start(out=gram_out[128:192, :], in_=gB[:])
        nc.sync.dma_start(out=kss_out[:], in_=kssb[:])

    nc.compile()
    return nc


def build_l2(R=64, W=128):
    NS = R * W
    nc = bacc.Bacc("TRN2", target_bir_lowering=False, debug=False, num_devices=8)
    v_in = nc.dram_tensor("v_in", [128, 2, NS], F8, kind="ExternalInput").ap()
    awT = nc.dram_tensor("awT", [128, 2, 192], F8, kind="ExternalInput").ap()
    pbias = nc.dram_tensor("pbias", [128, 2], F32, kind="ExternalInput").ap()
    sa = nc.dram_tensor("sa", [1, 1], F32, kind="ExternalInput").ap()
    out = nc.dram_tensor("out", [192, NS], F32, kind="ExternalOutput").ap()

    with tile.TileContext(nc) as tc, ExitStack() as ctx:
        wpool = ctx.enter_context(tc.tile_pool(name="w", bufs=1))
        vpool = ctx.enter_context(tc.tile_pool(name="v", bufs=4))
        opool = ctx.enter_context(tc.tile_pool(name="o", bufs=4))
        pspool = ctx.enter_context(tc.tile_pool(name="ps", bufs=4, space="PSUM"))

        aw = wpool.tile([128, 2, 192], F8)
        nc.sync.dma_start(out=aw[:], in_=awT[:])
        pb = wpool.tile([128, 2], F32)
        nc.sync.dma_start(out=pb[:], in_=pbias[:])
        sav = wpool.tile([128, 1], F32)
        nc.sync.dma_start(out=sav[:], in_=_bcast(sa[0:1, :], 128))

        for c0 in range(0, NS, 512):
            vt = vpool.tile([128, 2, 512], F8, tag="vt")
            nc.sync.dma_start(out=vt[:], in_=v_in[:, :, c0:c0 + 512])
            for mb in range(2):
                msz = 128 if mb == 0 else 64
                ps = pspool.tile([128, 512], F32, tag="ps")
                nc.tensor.matmul(ps[:msz, :], aw[:, :, mb * 128:mb * 128 + msz],
                                 vt[:], start=True, stop=True, perf_mode=DR)
                ot = opool.tile([128, 512], F32, tag="ot")
                nc.scalar.activation(out=ot[:msz, :], in_=ps[:msz, :],
                                     func=mybir.ActivationFunctionType.Identity,
                                     bias=pb[:msz, mb:mb + 1], scale=sav[:msz, 0:1])
                nc.sync.dma_start(out=out[mb * 128:mb * 128 + msz, c0:c0 + 512],
                                  in_=ot[:msz, :])
    nc.compile()
    return nc


WP = 130


def _dw_channel_at(pb, p):
    """kv channel (0..383) stored at partition p of dw pblock pb."""
    for (ipb, ib, opb, ob) in DWBLK:
        if opb == pb and ob <= p < ob + 64:
            return 128 * ipb + ib + (p - ob)
    return None


def prep_weights(w):
    """w: dict of reference weights (numpy f32). Returns dict of L1 input arrays."""
    out = {}
    qw = w["q_w"][:, :, 0, 0]          # (576, 192)
    qwT = np.zeros((128, 2, 768), np.float32)
    for (pb, h, x1b, qb, ch0, nch) in qkv_halves():
        win = 128 * pb + 64 * h
        qwT[0:128, 0, win:win + nch] = qw.T[0:128, ch0:ch0 + nch]
        qwT[0:64, 1, win:win + nch] = qw.T[128:192, ch0:ch0 + nch]
    out["qwT"] = qf8(qwT, SW)

    qdw = w["qdw_w"]                   # (576, 3, 3, 3) out, in-per-group, ky, kx
    qdwT = np.zeros((128, 54, 128), np.float32)
    for dy in (-1, 0, 1):
        for dx in (-1, 0, 1):
            ti = 3 * dy + dx + 4
            for (pb, h, x1b, qb, ch0, nch) in qkv_halves():
                for gl in range(nch // 3):
                    for i in range(3):
                        for j in range(3):
                            qdwT[x1b + 3 * gl + i, 6 * ti + pb, qb + 3 * gl + j] = \
                                qdw[ch0 + 3 * gl + j, i, dy + 1, dx + 1]
    out["qdw_wT"] = qdwT.astype(bf16)

    kvw = w["kv_w"]                    # (384, 192, 3, 3)
    # lo taps (128-ch rows) and packed hi taps, as DoubleRow pairs
    lo = np.zeros((9, 128, 384), np.float32)
    for dy in (-1, 0, 1):
        for dx in (-1, 0, 1):
            ti = 3 * dy + dx + 4
            lo[ti] = kvw[:, 0:128, dy + 1, dx + 1].T
    hi7 = np.zeros((128, 384), np.float32)
    hi7[0:64] = kvw[:, :, 2, 1].T[128:192]           # tap (1, 0) hi block
    kvp = np.zeros((4, 128, 384), np.float32)
    for j, dx in enumerate((-1, 0, 1)):
        kvp[j, 0:64] = kvw[:, :, 0, dx + 1].T[128:192]    # (-1, dx)
        kvp[j, 64:128] = kvw[:, :, 1, dx + 1].T[128:192]  # (0, dx)
    kvp[3, 0:64] = kvw[:, :, 2, 0].T[128:192]     # (1, -1)
    kvp[3, 64:128] = kvw[:, :, 2, 2].T[128:192]   # (1, +1)
    cwa = np.zeros((128, 7, 2, 384), np.float32)
    for i, (a, b) in enumerate([(0, 1), (2, 3), (4, 5), (6, 7)]):
        cwa[:, i, 0] = lo[a]
        cwa[:, i, 1] = lo[b]
    cwa[:, 4, 0] = lo[8]
    cwa[:, 4, 1] = hi7
    cwa[:, 5, 0] = kvp[0]
    cwa[:, 5, 1] = kvp[1]
    cwa[:, 6, 0] = kvp[2]
    cwa[:, 6, 1] = kvp[3]
    out["cw"] = qf8(cwa, SW)

    kvdw = w["kvdw_w"][:, 0]           # (384, 3, 3)
    dwT = np.zeros((128, 54, 64), np.float32)
    for dy in (-1, 0, 1):
        for dx in (-1, 0, 1):
            ti = 3 * dy + dx + 4
            d = kvdw[:, dy + 1, dx + 1]
            for blk, (ipb, ib, opb, ob) in enumerate(DWBLK):
                ch0 = 128 * ipb + ib
                dwT[ib:ib + 64, 6 * ti + blk, :] = np.diag(d[ch0:ch0 + 64])
    out["kvdw_wT"] = dwT.astype(bf16)

    # newk: pass0 = (qkv pb2, qkv pb3) @ SKE/SQKV; pass1 = (dw pb0, dw pb1-hi) @ SKE/SDW8
    nk = w["newk_w"][:, :, 0, 0]       # (192, 384): in = [k(192) | k_mask(192)]
    nkm = np.zeros((128, 2, 2, 192), np.float32)
    for pl, pb in enumerate((2, 3)):
        for p in range(128):
            ch = qkv_channel_at(pb, p)
            if ch is not None:
                nkm[p, 0, pl] = nk[:, ch - 192]          # k part: qkv ch 192-383
    for pl, pb in enumerate((0, 1)):
        for p in range(128):
            ch = _dw_channel_at(pb, p)
            if ch is not None and ch < 192:
                nkm[p, 1, pl] = nk[:, 192 + ch]          # k_mask: dw ch 0-191
    nkm[:, 0] *= SKE / SQKV / SW
    nkm[:, 1] *= SKE / SDW8 / SW
    out["nkw"] = qf8(nkm, SW)

    nv = w["newv_w"][:, :, 0, 0]       # (192, 384): in = [v(192) | v_mask(192)]
    nvm = np.zeros((128, 2, 2, 192), np.float32)
    for pl, pb in enumerate((4, 5)):
        for p in range(128):
            ch = qkv_channel_at(pb, p)
            if ch is not None:
                nvm[p, 0, pl] = nv[:, ch - 384]          # v part: qkv ch 384-575
    for pl, pb in enumerate((1, 2)):
        for p in range(128):
            ch = _dw_channel_at(pb, p)
            if ch is not None and ch >= 192:
                nvm[p, 1, pl] = nv[:, ch]                # v_mask: dw ch 192-383
    nvm[:, 0] *= SKE / SQKV / SW
    nvm[:, 1] *= SKE / SDW8 / SW
    out["nvw"] = qf8(nvm, SW)

    out["ident"] = np.eye(128, dtype=e4m3)
    out["ones_col"] = np.ones((128, 1), dtype=bf16)

    x1b = np.zeros((128, 6), np.float32)
    qkvb = np.zeros((128, 6), np.float32)
    for (pb, h, x1b_base, qb, ch0, nch) in qkv_halves():
        x1b[x1b_base:x1b_base + nch, pb] = w["q_b"][ch0:ch0 + nch]
        qkvb[qb:qb + nch, pb] = w["qdw_b"][ch0:ch0 + nch]
    out["x1_bias"] = x1b * (SXI * SW)
    out["qkv_bias"] = qkvb * SQKV

    kvb = np.zeros((128, 3), np.float32)
    kvb[:, 0] = w["kv_b"][0:128]
    kvb[0:64, 1] = w["kv_b"][128:192]
    kvb[64:128, 1] = w["kv_b"][192:256]
    kvb[:, 2] = w["kv_b"][256:384]
    out["kv_bias"] = kvb * (SXI * SW)
    dwb = np.zeros((128, 3), np.float32)
    for (ipb, ib, opb, ob) in DWBLK:
        dwb[ob:ob + 64, opb] = w["kvdw_b"][128 * ipb + ib:128 * ipb + ib + 64]
    out["kvdw_bias"] = dwb * SDW8
    out["newk_b_row"] = w["newk_b"][None, :].astype(np.float32) * SK8
    nvb = np.zeros((128, 2), np.float32)
    nvb[:, 0] = w["newv_b"][0:128]
    nvb[0:64, 1] = w["newv_b"][128:192]
    out["newv_bias"] = nvb * SV8
    return out


def prep_masks(R, H, half):
    m = np.zeros((R + 2, WP), np.float32)
    for r in range(R + 2):
        g = half * R + (r - 1)
        if 0 <= g < H:
            m[r, 1:129] = MINV
    return m.reshape(1, -1)


def prep_core(x, xm, b, half, R, H):
    xp = np.zeros((192, R + 2, WP), np.float32)
    mp = np.zeros((192, R + 4, WP), np.float32)
    for r in range(R + 2):
        g = half * R + (r - 1)
        if 0 <= g < H:
            xp[:, r, 1:129] = x[b, :, g, :]
    for r in range(R + 4):
        g = half * R + (r - 2)
        if 0 <= g < H:
            mp[:, r, 1:129] = xm[b, :, g, :]
    xp = xp.reshape(192, -1)
    xcb = np.zeros((128, 2, xp.shape[1]), np.float32)
    xcb[:, 0] = xp[0:128]
    xcb[0:64, 1] = xp[128:192]
    mp = mp.reshape(192, -1)
    L = mp.shape[1]
    hi = mp[128:192]
    d1 = np.zeros((128, L), np.float32)
    d2 = np.zeros((128, L), np.float32)
    d1[0:64] = hi
    d1[64:128, :L - 130] = hi[:, 130:]
    d2[0:64] = hi
    d2[64:128, :L - 2] = hi[:, 2:]
    return {
        "xc": qf8(xcb, SXI),
        "xm_lo": qf8(mp[0:128], SXI),
        "xm_d1": qf8(d1, SXI), "xm_d2": qf8(d2, SXI),
        "mask_rc": prep_masks(R, H, half).astype(bf16),
    }


def _q_maps():
    """q channel c (0..191) -> (pblock 0/1, partition)."""
    part = np.zeros(192, np.int64)
    pblk = np.zeros(192, np.int64)
    for (pb, h, x1b, qb, ch0, nch) in qkv_halves():
        if pb >= 2:
            continue
        for i in range(nch):
            pblk[ch0 + i] = pb
            part[ch0 + i] = qb + i
    return pblk, part


def _ss_from_qstats(stats, n_half):
    pblk, part = _q_maps()
    mv = stats.astype(np.float64)
    return (mv[part, pblk, 1] + mv[part, pblk, 0] ** 2) * n_half


def _ss_from_vstats(stats, n_half):
    ss = np.zeros(192, np.float64)
    mv = stats.astype(np.float64)
    ss[0:128] = (mv[0:128, 0, 1] + mv[0:128, 0, 0] ** 2) * n_half
    ss[128:192] = (mv[0:64, 1, 1] + mv[0:64, 1, 0] ** 2) * n_half
    return ss


def glue(res0, res1, temperature, proj_w, proj_b, n_half):
    """Combine two half-core L1 results -> L2 inputs (awT fp8, pbias, sa)."""
    G = res0["gram_out"].astype(np.float64) + res1["gram_out"].astype(np.float64)
    pblk, part = _q_maps()
    qcol = pblk * 128 + part
    G = G[:, qcol]                              # (d, c): sum_n k[d,n] q[c,n]
    qss = _ss_from_qstats(res0["qstats_out"], n_half) + _ss_from_qstats(res1["qstats_out"], n_half)
    vss = _ss_from_vstats(res0["vstats_out"], n_half) + _ss_from_vstats(res1["vstats_out"], n_half)
    kss = (res0["kss_out"].astype(np.float64) + res1["kss_out"].astype(np.float64))[0]
    qn = np.maximum(np.sqrt(qss), 1e-12)
    kn = np.maximum(np.sqrt(kss), 1e-12)
    vn = np.maximum(np.sqrt(vss), 1e-12)
    A = G.T / (qn[:, None] * kn[None, :])      # (c, d)
    M = np.zeros((192, 192), np.float64)
    t = np.asarray(temperature).reshape(-1)
    for h in range(8):
        sl = slice(24 * h, 24 * h + 24)
        a = A[sl, sl] * t[h]
        a = a - a.max(axis=-1, keepdims=True)
        e = np.exp(a)
        sm = e / e.sum(axis=-1, keepdims=True)
        M[sl, sl] = sm / vn[None, sl]
    At = proj_w[:, :, 0, 0].astype(np.float64) @ M   # (out-ch o, d)
    SA = 2.0 ** np.floor(np.log2(128.0 / max(np.abs(At).max(), 1e-30)))
    awT = np.zeros((128, 2, 192), np.float32)
    awT[:, 0, :] = At.T[0:128]
    awT[0:64, 1, :] = At.T[128:192]
    pbias = np.zeros((128, 2), np.float32)
    pbias[:, 0] = proj_b[0:128]
    pbias[0:64, 1] = proj_b[128:192]
    return {"awT": qf8(awT, SA), "pbias": pbias,
            "sa": np.full((1, 1), 1.0 / SA, np.float32)}


# ---------------- driver: kernel(**inputs) ----------------
from concourse.bass_utils import run_bass_kernel_spmd

R_FULL, H_FULL, B_FULL = 64, 128, 4
_NC1 = None
_NC2 = None


def _get_progs():
    global _NC1, _NC2
    if _NC1 is None:
        _NC1 = build_l1(R=R_FULL, S=16)
        _NC2 = build_l2(R=R_FULL)
    return _NC1, _NC2


def kernel(**inputs):
    inputs = {k: np.asarray(v) for k, v in inputs.items()}
    x, xm = inputs["x"], inputs["x_mask"]
    nc1, nc2 = _get_progs()
    wprep = prep_weights(inputs)
    in_maps = []
    for core in range(8):
        b, half = core // 2, core % 2
        m = dict(wprep)
        m.update(prep_core(x, xm, b, half, R_FULL, H_FULL))
        in_maps.append(m)
    res1 = run_bass_kernel_spmd(nc1, in_maps, list(range(8))).results

    n_half = R_FULL * 128
    in_maps2 = []
    for core in range(8):
        b, half = core // 2, core % 2
        if half == 0:
            l2c = glue(res1[2 * b], res1[2 * b + 1], inputs["temperature"],
                       inputs["proj_w"], inputs["proj_b"], n_half)
        m = dict(l2c)
        vv = np.array(res1[core]["v_out"])
        vv[64:128, 1, :] = np.zeros(1, e4m3)
        m["v_in"] = vv
        in_maps2.append(m)
    res2 = run_bass_kernel_spmd(nc2, in_maps2, list(range(8))).results

    out = np.empty((B_FULL, 192, H_FULL, 128), np.float32)
    for core in range(8):
        b, half = core // 2, core % 2
        out[b, :, half * R_FULL:(half + 1) * R_FULL, :] = \
            res2[core]["out"].reshape(192, R_FULL, 128)
    return out


# revision 12
# speedup vs baseline: 1.5630x; 1.0071x over previous
"""Trainium2 Bass kernel for nn_Cross_Attention (sparse_attention, 8 cores).

fp8(e4m3)+DoubleRow version: phases A/C/E/F and L2 run fp8 DoubleRow matmuls
(two 128-deep contraction chunks per pass at ~0.57 cyc/col); B/D keep bf16
quadrant matmuls but emit fp8. Power-of-2 scale bookkeeping throughout.
Shards 4 samples x 2 row-halves across 8 NeuronCores, glues partials on host.
"""
import sys
sys.path.insert(0, "/opt/trn_rl_repo")
import numpy as np
import ml_dtypes

import concourse.bass as bass
import concourse.tile as tile
from concourse import bacc, mybir
from contextlib import ExitStack


BF16 = mybir.dt.bfloat16
F8 = mybir.dt.float8e4
F32 = mybir.dt.float32
bf16 = ml_dtypes.bfloat16
e4m3 = ml_dtypes.float8_e4m3
DR = mybir.MatmulPerfMode.DoubleRow
DRI = mybir.MatmulPerfMode.DoubleRowSwInterleave

TAPS = [(dy, dx) for dy in (-1, 0, 1) for dx in (-1, 0, 1)]

# power-of-2 scales
SXI = 16.0        # x, x_mask fp8 pre-scale (2^4)
SW = 1024.0       # conv-weight fp8 pre-scale (2^10)
MINV = 1.0 / (SXI * SW)   # mask value de-scaling x1/kv1 back to true (2^-14)
SQKV = 512.0      # qkv fp8 storage scale (2^9)
SDW8 = 256.0      # dw fp8 storage scale (2^8)
SK8 = 512.0       # kT fp8 storage scale (2^9)
SV8 = 512.0       # v fp8 storage scale (2^9)
SKE = 2.0 ** 19   # newk/newv psum scale


def qf8(a, s):
    return np.clip(np.asarray(a, np.float32) * s, -240, 240).astype(e4m3)


def dri_pack(w2m):
    """[128, 2, M] -> [128, 2M] DoubleRowSwInterleave layout (pairs, reversed)."""
    a, b = w2m[:, 0], w2m[:, 1]
    inter = np.empty((w2m.shape[0], 2 * w2m.shape[2]), np.float32)
    inter[:, 0::2] = a[:, ::-1]
    inter[:, 1::2] = b[:, ::-1]
    return inter


def qkv_halves():
    """Per (pb, half): (x1_base, qkv_base, ch0, nch).  ch0 = qkv-global channel."""
    out = []
    for pb in range(6):
        P, odd = pb // 2, pb % 2
        for h in (0, 1):
            nch = 3 if (odd and h == 1) else 63
            ch0 = 3 * (64 * P + 42 * odd + 21 * h)
            x1b = 64 * h
            qb = 64 * h if not odd else 64 * (1 - h)
            out.append((pb, h, x1b, qb, ch0, nch))
    return out


def qkv_channel_at(pb, p):
    """qkv-global channel stored at partition p of qkv pblock pb, or None."""
    for (pb2, h, x1b, qb, ch0, nch) in qkv_halves():
        if pb2 == pb and qb <= p < qb + nch:
            return ch0 + (p - qb)
    return None


# newk/newv input chunk pairs (DoubleRow planes), see prep_weights
# kvdw 64-blocks: (in_pb, in_base, out_pb, out_base); dw pb1 halves swapped
DWBLK = [(0, 0, 0, 0), (0, 64, 0, 64), (1, 0, 1, 64),
         (1, 64, 1, 0), (2, 0, 2, 0), (2, 64, 2, 64)]


def _bcast(ap, p):
    return bass.AP(tensor=ap.tensor, offset=ap.offset, ap=[[0, p]] + list(ap.ap[1:]))


def _dr3(tile_ap, base, delta, cs):
    """[P, 2, cs] DoubleRow rhs view of a tile at element offset base,
    plane delta `delta`."""
    return bass.AP(tensor=tile_ap.tensor, offset=tile_ap.offset + base,
                   ap=[list(tile_ap.ap[0]), [delta, 2], [1, cs]])


def build_l1(R=64, W=128, S=16):
    assert W == 128 and R % S == 0 and S % 4 == 0
    WP = W + 2  # 130
    NSLAB = R // S
    NS = S * W
    XCOLS = (S + 2) * WP
    MCOLS = (S + 4) * WP
    MCW = MCOLS + 2
    N128 = NS // 128

    nc = bacc.Bacc("TRN2", target_bir_lowering=False, debug=False, num_devices=8)

    def din(name, shape, dt=F8):
        return nc.dram_tensor(name, shape, dt, kind="ExternalInput").ap()

    def dout(name, shape, dt=F32):
        return nc.dram_tensor(name, shape, dt, kind="ExternalOutput").ap()

    xc = din("xc", [128, 2, (R + 2) * WP])
    xm_lo = din("xm_lo", [128, (R + 4) * WP])
    xm_d1 = din("xm_d1", [128, (R + 4) * WP])
    xm_d2 = din("xm_d2", [128, (R + 4) * WP])
    qwT = din("qwT", [128, 6, 256])            # SwInterleave-packed per pblock
    qdw_wT = din("qdw_wT", [128, 9 * 6, 128], BF16)
    cw = din("cw", [128, 7, 3, 256])           # kv 3x3 tap-pair weights (interleaved)
    kvdw_wT = din("kvdw_wT", [128, 9 * 6, 64], BF16)
    nkw = din("nkw", [128, 2, 2, 192])
    nvw = din("nvw", [128, 2, 2, 256])
    ident = din("ident", [128, 128])
    ones_col = din("ones_col", [128, 1], BF16)
    x1_bias = din("x1_bias", [128, 6], F32)
    qkv_bias = din("qkv_bias", [128, 6], F32)
    kv_bias = din("kv_bias", [128, 3], F32)
    kvdw_bias = din("kvdw_bias", [128, 3], F32)
    newk_b_row = din("newk_b_row", [1, 192], F32)
    newv_bias = din("newv_bias", [128, 2], F32)
    mask_rc = din("mask_rc", [1, (R + 2) * WP], BF16)

    v_out = dout("v_out", [128, 2, R * W], F8)
    gram_out = dout("gram_out", [192, 256])
    kss_out = dout("kss_out", [1, 192])
    qstats_out = dout("qstats_out", [128, 2, 2])
    vstats_out = dout("vstats_out", [128, 2, 2])

    with tile.TileContext(nc) as tc, ExitStack() as ctx:
        wpool = ctx.enter_context(tc.tile_pool(name="weights", bufs=1))
        xpool = ctx.enter_context(tc.tile_pool(name="xslab", bufs=2))
        bigpool = ctx.enter_context(tc.tile_pool(name="big", bufs=1))
        midpool = ctx.enter_context(tc.tile_pool(name="mid", bufs=2))
        smpool = ctx.enter_context(tc.tile_pool(name="small", bufs=4))
        statpool = ctx.enter_context(tc.tile_pool(name="stats", bufs=1))
        pspool = ctx.enter_context(tc.tile_pool(name="ps", bufs=5, space="PSUM"))
        pspers = ctx.enter_context(tc.tile_pool(name="pspers", bufs=1, space="PSUM"))

        def load1(ap_in, shape, dt=F8, eng=None):
            t = wpool.tile(shape, dt, tag=ap_in.tensor.name)
            (eng or nc.sync).dma_start(out=t[:ap_in.shape[0]], in_=ap_in[:])
            return t

        # phase-A-critical constants on the sync queue; the rest via gpsimd queue
        qwT_s = load1(qwT, [128, 6, 256])
        x1b_s = load1(x1_bias, [128, 6], F32)
        g = nc.gpsimd
        qdw_s = load1(qdw_wT, [128, 9 * 6, 128], BF16, eng=g)
        cw_s = load1(cw, [128, 7, 3, 256], eng=g)
        kvdw_s = load1(kvdw_wT, [128, 9 * 6, 64], BF16, eng=g)
        nkw_s = load1(nkw, [128, 2, 2, 192], eng=g)
        nvw_s = load1(nvw, [128, 2, 2, 256], eng=g)
        id_s = load1(ident, [128, 128], eng=g)
        ones_s = load1(ones_col, [128, 1], BF16, eng=g)
        qkvb_s = load1(qkv_bias, [128, 6], F32, eng=g)
        kvb_s = load1(kv_bias, [128, 3], F32, eng=g)
        dwb_s = load1(kvdw_bias, [128, 3], F32, eng=g)
        nvb_s = load1(newv_bias, [128, 2], F32, eng=g)
        nkb_bc = wpool.tile([128, 192], F32, tag="nkb_bc")
        nc.gpsimd.dma_start(out=nkb_bc[:], in_=_bcast(newk_b_row[0:1, :], 128))

        gramA = pspers.tile([128, 256], F32)
        gramB = pspers.tile([64, 256], F32)
        kss_ps = pspers.tile([1, 192], F32)

        qstats = statpool.tile([128, 2, NSLAB * (NS // 512), 6], F32)
        vstats = statpool.tile([128, 2, NSLAB * (NS // 512), 6], F32)

        n128_total = NSLAB * N128

        for s in range(NSLAB):
            xsl = xpool.tile([128, 2, XCOLS], F8, tag="xsl")
            mc = xpool.tile([128, 3, MCW], F8, tag="mc")
            off = s * S * WP
            nc.sync.dma_start(out=xsl[:], in_=xc[:, :, off:off + XCOLS])
            nc.sync.dma_start(out=mc[:, 0, 1:1 + MCOLS], in_=xm_lo[:, off:off + MCOLS])
            nc.sync.dma_start(out=mc[:, 1, 1:1 + MCOLS], in_=xm_d1[:, off:off + MCOLS])
            nc.sync.dma_start(out=mc[:, 2, 1:1 + MCOLS], in_=xm_d2[:, off:off + MCOLS])
            for pl in range(3):
                nc.vector.memset(mc[:, pl, 0:1], 0.0)
                nc.vector.memset(mc[:, pl, MCW - 1:MCW], 0.0)
            mtile = xpool.tile([128, XCOLS], BF16, tag="mtile")
            nc.sync.dma_start(out=mtile[:], in_=_bcast(mask_rc[0:1, off:off + XCOLS], 128))

            # ---- Phase A: x1 = 1x1(x): one fp8 DoubleRow pass per (pb, c0)
            x1 = bigpool.tile([128, 6, XCOLS], BF16, tag="x1")
            for pb in range(6):
                for c0 in range(0, XCOLS, 512):
                    cs = min(512, XCOLS - c0)
                    ps = pspool.tile([128, 512], F32, tag="ps", name=f"psA{s}_{pb}_{c0}")
                    nc.tensor.matmul(
                        ps[:, :cs],
                        qwT_s[:, pb, :].rearrange("p (m two) -> p two m", two=2),
                        xsl[:, :, c0:c0 + cs], start=True, stop=True, perf_mode=DRI)
                    nc.vector.scalar_tensor_tensor(
                        out=x1[:, pb, c0:c0 + cs], in0=ps[:, :cs],
                        scalar=x1b_s[:, pb:pb + 1], in1=mtile[:, c0:c0 + cs],
                        op0=mybir.AluOpType.add, op1=mybir.AluOpType.mult)

            # ---- Phase C: kv1 = 3x3(xm): 7 fp8 DoubleRow tap-pair passes
            # pair rhs (base offset into mc, plane delta):
            CPASS = [(0, 1), (2, WP - 2), (WP + 1, 1), (2 * WP, 1),
                     (2 * WP + 2, MCW - 1),
                     (MCW, 1), (MCW + 2, MCW + 2 * WP - 2)]
            kv1 = bigpool.tile([128, 3, XCOLS], BF16, tag="kv1")
            for c0 in range(0, XCOLS, 512):
                cs = min(512, XCOLS - c0)
                for pb in range(3):
                    ps = pspool.tile([128, 512], F32, tag="ps", name=f"psC{s}_{pb}_{c0}")
                    for i, (base, delta) in enumerate(CPASS):
                        nc.tensor.matmul(
                            ps[:, :cs],
                            cw_s[:, i, pb, :].rearrange("p (m two) -> p two m", two=2),
                            _dr3(mc[:, 0, :], base + c0, delta, cs),
                            start=(i == 0), stop=(i == 6), perf_mode=DRI)
                    nc.vector.scalar_tensor_tensor(
                        out=kv1[:, pb, c0:c0 + cs], in0=ps[:, :cs],
                        scalar=kvb_s[:, pb:pb + 1], in1=mtile[:, c0:c0 + cs],
                        op0=mybir.AluOpType.add, op1=mybir.AluOpType.mult)

            # ---- Phase B: qkv = qdw(x1): pblock pairs, 4 concurrent 64x64 blocks
            qkv = bigpool.tile([128, 6, NS], F8, tag="qkv")
            x1v = [x1[:, pb, :].rearrange("p (r c) -> p r c", c=WP) for pb in range(6)]
            for sset in range(3):
                for ic in range(S // 4):
                    r0 = ic * 4
                    pse = pspool.tile([128, 512], F32, tag="ps", name=f"psBe{s}_{sset}_{ic}")
                    pso = pspool.tile([128, 512], F32, tag="ps", name=f"psBo{s}_{sset}_{ic}")
                    for ti, (dy, dx) in enumerate(TAPS):
                        for j in (0, 1):
                            pb = 2 * sset + j
                            pst = pse if j == 0 else pso
                            for h in (0, 1):
                                qb = 64 * h if j == 0 else 64 * (1 - h)
                                rhs = x1v[pb][64 * h:64 * h + 64,
                                              r0 + 1 + dy:r0 + 5 + dy, 1 + dx:1 + dx + 128]
                                nc.tensor.matmul(
                                    pst[qb:qb + 64, :],
                                    qdw_s[64 * h:64 * h + 64, 6 * ti + pb, qb:qb + 64],
                                    rhs, start=(ti == 0), stop=(ti == 8),
                                    tile_position=(64 * h, qb), skip_group_check=True)
                    for j in (0, 1):
                        pb = 2 * sset + j
                        nc.scalar.activation(out=qkv[:, pb, r0 * 128:(r0 + 4) * 128],
                                             in_=(pse if j == 0 else pso)[:, :],
                                             func=mybir.ActivationFunctionType.Identity,
                                             bias=qkvb_s[:, pb:pb + 1], scale=SQKV)

            # ---- Phase D: kvdw via 6 concurrent 64x64 diag blocks
            dw = bigpool.tile([128, 3, NS], F8, tag="dw")
            kv1v = [kv1[:, pb, :].rearrange("p (r c) -> p r c", c=WP) for pb in range(3)]
            for ic in range(S // 4):
                r0 = ic * 4
                pss = [pspool.tile([128, 512], F32, tag="ps", name=f"psD{s}_{ic}_{i}")
                       for i in range(3)]
                for ti, (dy, dx) in enumerate(TAPS):
                    for blk, (ipb, ib, opb, ob) in enumerate(DWBLK):
                        rhs = kv1v[ipb][ib:ib + 64,
                                        r0 + 1 + dy:r0 + 5 + dy, 1 + dx:1 + dx + 128]
                        nc.tensor.matmul(pss[opb][ob:ob + 64, :],
                                         kvdw_s[ib:ib + 64, 6 * ti + blk, :],
                                         rhs, start=(ti == 0), stop=(ti == 8),
                                         tile_position=(ib, ob), skip_group_check=True)
                for pb in range(3):
                    nc.scalar.activation(out=dw[:, pb, r0 * 128:(r0 + 4) * 128],
                                         in_=pss[pb][:, :],
                                         func=mybir.ActivationFunctionType.Identity,
                                         bias=dwb_s[:, pb:pb + 1], scale=SDW8)

            # ---- Phase E: v = newv(v_cc): 2 fp8 DoubleRow passes per (c0, mb)
            vt = midpool.tile([128, 2, NS], F8, tag="vt")
            for ic in range(NS // 512):
                c0 = ic * 512
                for mb in range(2):
                    msz = 128 if mb == 0 else 64
                    ps = pspool.tile([128, 512], F32, tag="ps", name=f"psE{s}_{ic}_{mb}")
                    nc.tensor.matmul(
                        ps[:, :],
                        nvw_s[:, 0, mb, :].rearrange("p (m two) -> p two m", two=2),
                        qkv[:, 4:6, c0:c0 + 512], start=True, stop=False, perf_mode=DRI)
                    nc.tensor.matmul(
                        ps[:, :],
                        nvw_s[:, 1, mb, :].rearrange("p (m two) -> p two m", two=2),
                        dw[:, 1:3, c0:c0 + 512], start=False, stop=True, perf_mode=DRI)
                    nc.scalar.activation(out=vt[:msz, mb, c0:c0 + 512], in_=ps[:msz, :],
                                         func=mybir.ActivationFunctionType.Identity,
                                         bias=nvb_s[:msz, mb:mb + 1], scale=SV8 / SKE)
            nc.sync.dma_start(out=v_out[:, 0, s * NS:(s + 1) * NS], in_=vt[:, 0, :])
            nc.sync.dma_start(out=v_out[0:64, 1, s * NS:(s + 1) * NS], in_=vt[:64, 1, :])
            for sub in range(NS // 512):
                si = s * (NS // 512) + sub
                sl = slice(sub * 512, (sub + 1) * 512)
                nc.vector.bn_stats(out=vstats[:, 0, si, :], in_=vt[:, 0, sl])
                nc.vector.bn_stats(out=vstats[:64, 1, si, :], in_=vt[:64, 1, sl])
                nc.vector.bn_stats(out=qstats[:, 0, si, :], in_=qkv[:, 0, sl])
                nc.vector.bn_stats(out=qstats[:, 1, si, :], in_=qkv[:, 1, sl])

            # ---- Phase F: per 256-px group: kT pair, qT pair, DoubleRow gram, kss
            for gi in range(N128 // 2):
                gidx = s * (N128 // 2) + gi
                kT2 = smpool.tile([128, 2, 192], F8, tag="kT2")
                # fp8 transpose requires output element step 2: interleaved psum
                qps = pspool.tile([128, 2, 512], F8, tag="ps", name=f"qps{s}_{gi}")
                qps_f = qps[:, 0, :]
                pstr = list(qps_f.ap[0])
                for ci in (0, 1):
                    c0 = (2 * gi + ci) * 128
                    kps = pspool.tile([128, 192], F32, tag="ps", name=f"kps{s}_{gi}_{ci}")
                    nc.tensor.matmul(kps[:, :], qkv[:, 2:4, c0:c0 + 128],
                                     nkw_s[:, 0, :, :], start=True, stop=False,
                                     perf_mode=DR)
                    nc.tensor.matmul(kps[:, :], dw[:, 0:2, c0:c0 + 128],
                                     nkw_s[:, 1, :, :], start=False, stop=True,
                                     perf_mode=DR)
                    nc.vector.scalar_tensor_tensor(
                        out=kT2[:, ci, :], in0=kps[:], scalar=SK8 / SKE, in1=nkb_bc[:],
                        op0=mybir.AluOpType.mult, op1=mybir.AluOpType.add)
                    for ch in (0, 1):
                        o2 = bass.AP(tensor=qps_f.tensor,
                                     offset=qps_f.offset + ci * 512 + ch * 256,
                                     ap=[pstr, [2, 128]])
                        nc.tensor.transpose(o2, qkv[:, ch, c0:c0 + 128], id_s[:, :])
                    ksq = smpool.tile([128, 192], BF16, tag="ksq")
                    nc.vector.tensor_mul(ksq[:], kT2[:, ci, :], kT2[:, ci, :])
                    nc.tensor.matmul(kss_ps[:, :], ones_s[:, :], ksq[:],
                                     start=(gidx == 0 and ci == 0),
                                     stop=(gidx == n128_total // 2 - 1 and ci == 1))
                qT2 = smpool.tile([128, 2, 256], F8, tag="qT2")
                qps_v = bass.AP(tensor=qps_f.tensor, offset=qps_f.offset,
                                ap=[pstr, [512, 2], [256, 2], [2, 128]])
                qT2_v = qT2[:].rearrange("p c (h n) -> p c h n", h=2)
                nc.scalar.copy(out=qT2_v, in_=qps_v)
                nc.tensor.matmul(gramA[:, :], kT2[:, :, 0:128], qT2[:],
                                 start=(gidx == 0), stop=(gidx == n128_total // 2 - 1),
                                 perf_mode=DR)
                nc.tensor.matmul(gramB[:, :], kT2[:, :, 128:192], qT2[:],
                                 start=(gidx == 0), stop=(gidx == n128_total // 2 - 1),
                                 perf_mode=DR)

        qmv = statpool.tile([128, 2, 2], F32)
        vmv = statpool.tile([128, 2, 2], F32)
        nc.vector.memset(qmv[:], 0.0)
        nc.vector.memset(vmv[:], 0.0)
        nc.vector.bn_aggr(out=qmv[:, 0, :], in_=qstats[:, 0, :, :])
        nc.vector.bn_aggr(out=qmv[:, 1, :], in_=qstats[:, 1, :, :])
        nc.vector.bn_aggr(out=vmv[:, 0, :], in_=vstats[:, 0, :, :])
        nc.vector.bn_aggr(out=vmv[:64, 1, :], in_=vstats[:64, 1, :, :])
        nc.sync.dma_start(out=qstats_out[:], in_=qmv[:])
        nc.sync.dma_start(out=vstats_out[:], in_=vmv[:])
        gA = statpool.tile([128, 256], F32)
        gB = statpool.tile([64, 256], F32)
        kssb = statpool.tile([1, 192], F32)
        nc.scalar.copy(out=gA[:], in_=gramA[:])
        nc.scalar.copy(out=gB[:], in_=gramB[:])
        nc.scalar.copy(out=kssb[:], in_=kss_ps[:])
        nc.sync.dma_start(out=gram_out[0:128, :], in_=gA[:])
        nc.sync.dma_start(out=gram_out[128:192, :], in_=gB[:])
        nc.sync.dma_start(out=kss_out[:], in_=kssb[:])

    nc.compile()
    return nc


def build_l2(R=64, W=128):
    NS = R * W
    nc = bacc.Bacc("TRN2", target_bir_lowering=False, debug=False, num_devices=8)
    v_in = nc.dram_tensor("v_in", [128, 2 * NS], F8, kind="ExternalInput").ap()
    awT = nc.dram_tensor("awT", [128, 2, 256], F8, kind="ExternalInput").ap()
    pbias = nc.dram_tensor("pbias", [128, 2], F32, kind="ExternalInput").ap()
    sab = nc.dram_tensor("sab", [128, 2], F32, kind="ExternalInput").ap()  # bias*SA
    out = nc.dram_tensor("out", [128, 2, NS], BF16, kind="ExternalOutput").ap()

    with tile.TileContext(nc) as tc, ExitStack() as ctx:
        wpool = ctx.enter_context(tc.tile_pool(name="w", bufs=1))
        vpool = ctx.enter_context(tc.tile_pool(name="v", bufs=1))
        opool = ctx.enter_context(tc.tile_pool(name="o", bufs=1))
        pspool = ctx.enter_context(tc.tile_pool(name="ps", bufs=7, space="PSUM"))

        aw = wpool.tile([128, 2, 256], F8)
        nc.sync.dma_start(out=aw[:], in_=awT[:])
        pb = wpool.tile([128, 2], F32)
        nc.sync.dma_start(out=pb[:], in_=pbias[:])
        sav = wpool.tile([128, 2], F32)
        nc.sync.dma_start(out=sav[:], in_=sab[:])
        vt = vpool.tile([128, 2, NS], F8)
        nc.sync.dma_start(out=vt[:], in_=v_in[:])
        ot = opool.tile([128, 2, NS], BF16)

        # mb-outer: consecutive matmuls share the stationary operand
        for mb in range(2):
            msz = 128 if mb == 0 else 64
            for ic in range(NS // 512):
                c0 = ic * 512
                ps = pspool.tile([128, 512], F32, tag="ps", name=f"ps{mb}_{ic}")
                nc.tensor.matmul(
                    ps[:, :],
                    aw[:, mb, :].rearrange("p (m two) -> p two m", two=2),
                    vt[:, :, c0:c0 + 512], start=True, stop=True, perf_mode=DRI)
                # out = (ps + SA*bias) * (1/SA), on the vector engine
                nc.vector.tensor_scalar(
                    out=ot[:msz, mb, c0:c0 + 512], in0=ps[:msz, :],
                    scalar1=pb[:msz, mb:mb + 1], scalar2=sav[:msz, mb:mb + 1],
                    op0=mybir.AluOpType.add, op1=mybir.AluOpType.mult)
            nc.sync.dma_start(out=out[:, mb, :], in_=ot[:, mb, :])
    nc.compile()
    return nc


WP = 130


def _dw_channel_at(pb, p):
    """kv channel (0..383) stored at partition p of dw pblock pb."""
    for (ipb, ib, opb, ob) in DWBLK:
        if opb == pb and ob <= p < ob + 64:
            return 128 * ipb + ib + (p - ob)
    return None


def prep_weights(w):
    """w: dict of reference weights (numpy f32). Returns dict of L1 input arrays."""
    out = {}
    qw = w["q_w"][:, :, 0, 0]          # (576, 192)
    qwT = np.zeros((128, 2, 768), np.float32)
    for (pb, h, x1b, qb, ch0, nch) in qkv_halves():
        win = 128 * pb + 64 * h
        qwT[0:128, 0, win:win + nch] = qw.T[0:128, ch0:ch0 + nch]
        qwT[0:64, 1, win:win + nch] = qw.T[128:192, ch0:ch0 + nch]
    qwTi = np.zeros((128, 6, 256), np.float32)
    for pb in range(6):
        qwTi[:, pb] = dri_pack(qwT[:, :, 128 * pb:128 * pb + 128])
    out["qwT"] = qf8(qwTi, SW)

    qdw = w["qdw_w"]                   # (576, 3, 3, 3) out, in-per-group, ky, kx
    qdwT = np.zeros((128, 54, 128), np.float32)
    for dy in (-1, 0, 1):
        for dx in (-1, 0, 1):
            ti = 3 * dy + dx + 4
            for (pb, h, x1b, qb, ch0, nch) in qkv_halves():
                for gl in range(nch // 3):
                    for i in range(3):
                        for j in range(3):
                            qdwT[x1b + 3 * gl + i, 6 * ti + pb, qb + 3 * gl + j] = \
                                qdw[ch0 + 3 * gl + j, i, dy + 1, dx + 1]
    out["qdw_wT"] = qdwT.astype(bf16)

    kvw = w["kv_w"]                    # (384, 192, 3, 3)
    # lo taps (128-ch rows) and packed hi taps, as DoubleRow pairs
    lo = np.zeros((9, 128, 384), np.float32)
    for dy in (-1, 0, 1):
        for dx in (-1, 0, 1):
            ti = 3 * dy + dx + 4
            lo[ti] = kvw[:, 0:128, dy + 1, dx + 1].T
    hi7 = np.zeros((128, 384), np.float32)
    hi7[0:64] = kvw[:, :, 2, 1].T[128:192]           # tap (1, 0) hi block
    kvp = np.zeros((4, 128, 384), np.float32)
    for j, dx in enumerate((-1, 0, 1)):
        kvp[j, 0:64] = kvw[:, :, 0, dx + 1].T[128:192]    # (-1, dx)
        kvp[j, 64:128] = kvw[:, :, 1, dx + 1].T[128:192]  # (0, dx)
    kvp[3, 0:64] = kvw[:, :, 2, 0].T[128:192]     # (1, -1)
    kvp[3, 64:128] = kvw[:, :, 2, 2].T[128:192]   # (1, +1)
    cwa = np.zeros((128, 7, 2, 384), np.float32)
    for i, (a, b) in enumerate([(0, 1), (2, 3), (4, 5), (6, 7)]):
        cwa[:, i, 0] = lo[a]
        cwa[:, i, 1] = lo[b]
    cwa[:, 4, 0] = lo[8]
    cwa[:, 4, 1] = hi7
    cwa[:, 5, 0] = kvp[0]
    cwa[:, 5, 1] = kvp[1]
    cwa[:, 6, 0] = kvp[2]
    cwa[:, 6, 1] = kvp[3]
    cwi = np.zeros((128, 7, 3, 256), np.float32)
    for i in range(7):
        for pb in range(3):
            cwi[:, i, pb] = dri_pack(cwa[:, i, :, 128 * pb:128 * pb + 128])
    out["cw"] = qf8(cwi, SW)

    kvdw = w["kvdw_w"][:, 0]           # (384, 3, 3)
    dwT = np.zeros((128, 54, 64), np.float32)
    for dy in (-1, 0, 1):
        for dx in (-1, 0, 1):
            ti = 3 * dy + dx + 4
            d = kvdw[:, dy + 1, dx + 1]
            for blk, (ipb, ib, opb, ob) in enumerate(DWBLK):
                ch0 = 128 * ipb + ib
                dwT[ib:ib + 64, 6 * ti + blk, :] = np.diag(d[ch0:ch0 + 64])
    out["kvdw_wT"] = dwT.astype(bf16)

    # newk: pass0 = (qkv pb2, qkv pb3) @ SKE/SQKV; pass1 = (dw pb0, dw pb1-hi) @ SKE/SDW8
    nk = w["newk_w"][:, :, 0, 0]       # (192, 384): in = [k(192) | k_mask(192)]
    nkm = np.zeros((128, 2, 2, 192), np.float32)
    for pl, pb in enumerate((2, 3)):
        for p in range(128):
            ch = qkv_channel_at(pb, p)
            if ch is not None:
                nkm[p, 0, pl] = nk[:, ch - 192]          # k part: qkv ch 192-383
    for pl, pb in enumerate((0, 1)):
        for p in range(128):
            ch = _dw_channel_at(pb, p)
            if ch is not None and ch < 192:
                nkm[p, 1, pl] = nk[:, 192 + ch]          # k_mask: dw ch 0-191
    nkm[:, 0] *= SKE / SQKV / SW
    nkm[:, 1] *= SKE / SDW8 / SW
    out["nkw"] = qf8(nkm, SW)

    nv = w["newv_w"][:, :, 0, 0]       # (192, 384): in = [v(192) | v_mask(192)]
    nvm = np.zeros((128, 2, 2, 192), np.float32)
    for pl, pb in enumerate((4, 5)):
        for p in range(128):
            ch = qkv_channel_at(pb, p)
            if ch is not None:
                nvm[p, 0, pl] = nv[:, ch - 384]          # v part: qkv ch 384-575
    for pl, pb in enumerate((1, 2)):
        for p in range(128):
            ch = _dw_channel_at(pb, p)
            if ch is not None and ch >= 192:
                nvm[p, 1, pl] = nv[:, ch]                # v_mask: dw ch 192-383
    nvm[:, 0] *= SKE / SQKV / SW
    nvm[:, 1] *= SKE / SDW8 / SW
    nvp = np.zeros((128, 2, 2, 2, 128), np.float32)
    nvp[:, :, :, 0, :] = nvm[:, :, :, 0:128]
    nvp[:, :, :, 1, 0:64] = nvm[:, :, :, 128:192]
    nvwi = np.zeros((128, 2, 2, 256), np.float32)
    for ps_ in range(2):
        for mb in range(2):
            nvwi[:, ps_, mb] = dri_pack(nvp[:, ps_, :, mb, :])
    out["nvw"] = qf8(nvwi, SW)

    out["ident"] = np.eye(128, dtype=e4m3)
    out["ones_col"] = np.ones((128, 1), dtype=bf16)

    x1b = np.zeros((128, 6), np.float32)
    qkvb = np.zeros((128, 6), np.float32)
    for (pb, h, x1b_base, qb, ch0, nch) in qkv_halves():
        x1b[x1b_base:x1b_base + nch, pb] = w["q_b"][ch0:ch0 + nch]
        qkvb[qb:qb + nch, pb] = w["qdw_b"][ch0:ch0 + nch]
    out["x1_bias"] = x1b * (SXI * SW)
    out["qkv_bias"] = qkvb * SQKV

    kvb = np.zeros((128, 3), np.float32)
    kvb[:, 0] = w["kv_b"][0:128]
    kvb[0:64, 1] = w["kv_b"][128:192]
    kvb[64:128, 1] = w["kv_b"][192:256]
    kvb[:, 2] = w["kv_b"][256:384]
    out["kv_bias"] = kvb * (SXI * SW)
    dwb = np.zeros((128, 3), np.float32)
    for (ipb, ib, opb, ob) in DWBLK:
        dwb[ob:ob + 64, opb] = w["kvdw_b"][128 * ipb + ib:128 * ipb + ib + 64]
    out["kvdw_bias"] = dwb * SDW8
    out["newk_b_row"] = w["newk_b"][None, :].astype(np.float32) * SK8
    nvb = np.zeros((128, 2), np.float32)
    nvb[:, 0] = w["newv_b"][0:128]
    nvb[0:64, 1] = w["newv_b"][128:192]
    out["newv_bias"] = nvb * SV8
    return out


def prep_masks(R, H, half):
    m = np.zeros((R + 2, WP), np.float32)
    for r in range(R + 2):
        g = half * R + (r - 1)
        if 0 <= g < H:
            m[r, 1:129] = MINV
    return m.reshape(1, -1)


def prep_core(x, xm, b, half, R, H):
    xp = np.zeros((192, R + 2, WP), np.float32)
    mp = np.zeros((192, R + 4, WP), np.float32)
    for r in range(R + 2):
        g = half * R + (r - 1)
        if 0 <= g < H:
            xp[:, r, 1:129] = x[b, :, g, :]
    for r in range(R + 4):
        g = half * R + (r - 2)
        if 0 <= g < H:
            mp[:, r, 1:129] = xm[b, :, g, :]
    xp = xp.reshape(192, -1)
    xcb = np.zeros((128, 2, xp.shape[1]), np.float32)
    xcb[:, 0] = xp[0:128]
    xcb[0:64, 1] = xp[128:192]
    mp = mp.reshape(192, -1)
    L = mp.shape[1]
    hi = mp[128:192]
    d1 = np.zeros((128, L), np.float32)
    d2 = np.zeros((128, L), np.float32)
    d1[0:64] = hi
    d1[64:128, :L - 130] = hi[:, 130:]
    d2[0:64] = hi
    d2[64:128, :L - 2] = hi[:, 2:]
    return {
        "xc": qf8(xcb, SXI),
        "xm_lo": qf8(mp[0:128], SXI),
        "xm_d1": qf8(d1, SXI), "xm_d2": qf8(d2, SXI),
        "mask_rc": prep_masks(R, H, half).astype(bf16),
    }


def _q_maps():
    """q channel c (0..191) -> (pblock 0/1, partition)."""
    part = np.zeros(192, np.int64)
    pblk = np.zeros(192, np.int64)
    for (pb, h, x1b, qb, ch0, nch) in qkv_halves():
        if pb >= 2:
            continue
        for i in range(nch):
            pblk[ch0 + i] = pb
            part[ch0 + i] = qb + i
    return pblk, part


def _ss_from_qstats(stats, n_half):
    pblk, part = _q_maps()
    mv = stats.astype(np.float64)
    return (mv[part, pblk, 1] + mv[part, pblk, 0] ** 2) * n_half


def _ss_from_vstats(stats, n_half):
    ss = np.zeros(192, np.float64)
    mv = stats.astype(np.float64)
    ss[0:128] = (mv[0:128, 0, 1] + mv[0:128, 0, 0] ** 2) * n_half
    ss[128:192] = (mv[0:64, 1, 1] + mv[0:64, 1, 0] ** 2) * n_half
    return ss


def glue(res0, res1, temperature, proj_w, proj_b, n_half):
    """Combine two half-core L1 results -> L2 inputs (awT fp8, pbias, sa)."""
    G = res0["gram_out"].astype(np.float64) + res1["gram_out"].astype(np.float64)
    pblk, part = _q_maps()
    qcol = pblk * 128 + part
    G = G[:, qcol]                              # (d, c): sum_n k[d,n] q[c,n]
    qss = _ss_from_qstats(res0["qstats_out"], n_half) + _ss_from_qstats(res1["qstats_out"], n_half)
    vss = _ss_from_vstats(res0["vstats_out"], n_half) + _ss_from_vstats(res1["vstats_out"], n_half)
    kss = (res0["kss_out"].astype(np.float64) + res1["kss_out"].astype(np.float64))[0]
    qn = np.maximum(np.sqrt(qss), 1e-12)
    kn = np.maximum(np.sqrt(kss), 1e-12)
    vn = np.maximum(np.sqrt(vss), 1e-12)
    A = G.T / (qn[:, None] * kn[None, :])      # (c, d)
    M = np.zeros((192, 192), np.float64)
    t = np.asarray(temperature).reshape(-1)
    for h in range(8):
        sl = slice(24 * h, 24 * h + 24)
        a = A[sl, sl] * t[h]
        a = a - a.max(axis=-1, keepdims=True)
        e = np.exp(a)
        sm = e / e.sum(axis=-1, keepdims=True)
        M[sl, sl] = sm / vn[None, sl]
    At = proj_w[:, :, 0, 0].astype(np.float64) @ M   # (out-ch o, d)
    SA = 2.0 ** np.floor(np.log2(128.0 / max(np.abs(At).max(), 1e-30)))
    awT = np.zeros((128, 2, 192), np.float32)
    awT[:, 0, :] = At.T[0:128]
    awT[0:64, 1, :] = At.T[128:192]
    awp = np.zeros((128, 2, 2, 128), np.float32)
    awp[:, :, 0, :] = awT[:, :, 0:128]
    awp[:, :, 1, 0:64] = awT[:, :, 128:192]
    awTi = np.zeros((128, 2, 256), np.float32)
    for mb in range(2):
        awTi[:, mb] = dri_pack(awp[:, :, mb, :])
    pbias = np.zeros((128, 2), np.float32)
    pbias[:, 0] = proj_b[0:128]
    pbias[0:64, 1] = proj_b[128:192]
    return {"awT": qf8(awTi, SA), "pbias": pbias * SA,
            "sab": np.full((128, 2), 1.0 / SA, np.float32)}


def _prep_vin(v_out):
    """L1 v_out [128, 2, NS] -> L2 v_in [128, 2*NS], pad rows zeroed."""
    vv = np.array(v_out)
    vv[64:128, 1, :] = np.zeros(1, e4m3)
    return vv.reshape(128, -1)


# ---------------- driver: kernel(**inputs) ----------------
from concourse.bass_utils import run_bass_kernel_spmd

R_FULL, H_FULL, B_FULL = 64, 128, 4
_NC1 = None
_NC2 = None


def _get_progs():
    global _NC1, _NC2
    if _NC1 is None:
        _NC1 = build_l1(R=R_FULL, S=16)
        _NC2 = build_l2(R=R_FULL)
    return _NC1, _NC2


def kernel(**inputs):
    inputs = {k: np.asarray(v) for k, v in inputs.items()}
    x, xm = inputs["x"], inputs["x_mask"]
    nc1, nc2 = _get_progs()
    wprep = prep_weights(inputs)
    in_maps = []
    for core in range(8):
        b, half = core // 2, core % 2
        m = dict(wprep)
        m.update(prep_core(x, xm, b, half, R_FULL, H_FULL))
        in_maps.append(m)
    res1 = run_bass_kernel_spmd(nc1, in_maps, list(range(8))).results

    n_half = R_FULL * 128
    in_maps2 = []
    for core in range(8):
        b, half = core // 2, core % 2
        if half == 0:
            l2c = glue(res1[2 * b], res1[2 * b + 1], inputs["temperature"],
                       inputs["proj_w"], inputs["proj_b"], n_half)
        m = dict(l2c)
        m["v_in"] = _prep_vin(res1[core]["v_out"])
        in_maps2.append(m)
    res2 = run_bass_kernel_spmd(nc2, in_maps2, list(range(8))).results

    out = np.empty((B_FULL, 192, H_FULL, 128), np.float32)
    for core in range(8):
        b, half = core // 2, core % 2
        o = np.asarray(res2[core]["out"]).astype(np.float32)   # [128, 2, NS] bf16
        sl = out[b, :, half * R_FULL:(half + 1) * R_FULL, :]
        sl[0:128] = o[:, 0, :].reshape(128, R_FULL, 128)
        sl[128:192] = o[0:64, 1, :].reshape(64, R_FULL, 128)
    return out


# revision 14
# speedup vs baseline: 1.6090x; 1.0294x over previous
"""Trainium2 Bass kernel for nn_Cross_Attention (sparse_attention, 8 cores).

fp8(e4m3)+DoubleRow version: phases A/C/E/F and L2 run fp8 DoubleRow matmuls
(two 128-deep contraction chunks per pass at ~0.57 cyc/col); B/D keep bf16
quadrant matmuls but emit fp8. Power-of-2 scale bookkeeping throughout.
Shards 4 samples x 2 row-halves across 8 NeuronCores, glues partials on host.
"""
import sys
sys.path.insert(0, "/opt/trn_rl_repo")
import numpy as np
import ml_dtypes

import concourse.bass as bass
import concourse.tile as tile
from concourse import bacc, mybir
from contextlib import ExitStack


BF16 = mybir.dt.bfloat16
F8 = mybir.dt.float8e4
F32 = mybir.dt.float32
bf16 = ml_dtypes.bfloat16
e4m3 = ml_dtypes.float8_e4m3
DR = mybir.MatmulPerfMode.DoubleRow
DRI = mybir.MatmulPerfMode.DoubleRowSwInterleave

TAPS = [(dy, dx) for dy in (-1, 0, 1) for dx in (-1, 0, 1)]

# power-of-2 scales
SXI = 16.0        # x, x_mask fp8 pre-scale (2^4)
SW = 1024.0       # conv-weight fp8 pre-scale (2^10)
MINV = 1.0 / (SXI * SW)   # mask value de-scaling x1/kv1 back to true (2^-14)
SQKV = 512.0      # qkv fp8 storage scale (2^9)
SDW8 = 256.0      # dw fp8 storage scale (2^8)
SK8 = 512.0       # kT fp8 storage scale (2^9)
SV8 = 512.0       # v fp8 storage scale (2^9)
SKE = 2.0 ** 19   # newk/newv psum scale


def qf8(a, s):
    return np.clip(np.asarray(a, np.float32) * s, -240, 240).astype(e4m3)


def dri_pack(w2m):
    """[128, 2, M] -> [128, 2M] DoubleRowSwInterleave layout (pairs, reversed)."""
    a, b = w2m[:, 0], w2m[:, 1]
    inter = np.empty((w2m.shape[0], 2 * w2m.shape[2]), np.float32)
    inter[:, 0::2] = a[:, ::-1]
    inter[:, 1::2] = b[:, ::-1]
    return inter


def qkv_halves():
    """Per (pb, half): (x1_base, qkv_base, ch0, nch).  ch0 = qkv-global channel."""
    out = []
    for pb in range(6):
        P, odd = pb // 2, pb % 2
        for h in (0, 1):
            nch = 3 if (odd and h == 1) else 63
            ch0 = 3 * (64 * P + 42 * odd + 21 * h)
            x1b = 64 * h
            qb = 64 * h if not odd else 64 * (1 - h)
            out.append((pb, h, x1b, qb, ch0, nch))
    return out


def qkv_channel_at(pb, p):
    """qkv-global channel stored at partition p of qkv pblock pb, or None."""
    for (pb2, h, x1b, qb, ch0, nch) in qkv_halves():
        if pb2 == pb and qb <= p < qb + nch:
            return ch0 + (p - qb)
    return None


# newk/newv input chunk pairs (DoubleRow planes), see prep_weights
# kvdw 64-blocks: (in_pb, in_base, out_pb, out_base); dw pb1 halves swapped
DWBLK = [(0, 0, 0, 0), (0, 64, 0, 64), (1, 0, 1, 64),
         (1, 64, 1, 0), (2, 0, 2, 0), (2, 64, 2, 64)]


def _bcast(ap, p):
    return bass.AP(tensor=ap.tensor, offset=ap.offset, ap=[[0, p]] + list(ap.ap[1:]))


def _dr3(tile_ap, base, delta, cs):
    """[P, 2, cs] DoubleRow rhs view of a tile at element offset base,
    plane delta `delta`."""
    return bass.AP(tensor=tile_ap.tensor, offset=tile_ap.offset + base,
                   ap=[list(tile_ap.ap[0]), [delta, 2], [1, cs]])


def build_l1(R=64, W=128, S=16):
    assert W == 128 and R % S == 0 and S % 4 == 0
    WP = W + 2  # 130
    NSLAB = R // S
    NS = S * W
    XCOLS = (S + 2) * WP
    MCOLS = (S + 4) * WP
    MCW = MCOLS + 2
    N128 = NS // 128

    nc = bacc.Bacc("TRN2", target_bir_lowering=False, debug=False, num_devices=8)

    def din(name, shape, dt=F8):
        return nc.dram_tensor(name, shape, dt, kind="ExternalInput").ap()

    def dout(name, shape, dt=F32):
        return nc.dram_tensor(name, shape, dt, kind="ExternalOutput").ap()

    xc = din("xc", [128, 2, (R + 2) * WP])
    xm_lo = din("xm_lo", [128, (R + 4) * WP])
    xm_d1 = din("xm_d1", [128, (R + 4) * WP])
    xm_d2 = din("xm_d2", [128, (R + 4) * WP])
    qwT = din("qwT", [128, 6, 256])            # SwInterleave-packed per pblock
    qdw_wT = din("qdw_wT", [128, 9 * 6, 128], BF16)
    cw = din("cw", [128, 7, 3, 256])           # kv 3x3 tap-pair weights (interleaved)
    kvdw_wT = din("kvdw_wT", [128, 9 * 6, 64], BF16)
    nkw = din("nkw", [128, 2, 2, 192])
    nvw = din("nvw", [128, 2, 2, 256])
    ident = din("ident", [128, 128])
    ones_col = din("ones_col", [128, 1], BF16)
    x1_bias = din("x1_bias", [128, 6], F32)
    qkv_bias = din("qkv_bias", [128, 6], F32)
    kv_bias = din("kv_bias", [128, 3], F32)
    kvdw_bias = din("kvdw_bias", [128, 3], F32)
    newk_b_row = din("newk_b_row", [1, 192], F32)
    newv_bias = din("newv_bias", [128, 2], F32)
    mask_rc = din("mask_rc", [1, (R + 2) * WP], BF16)

    v_out = dout("v_out", [128, 2, R * W], F8)
    gram_out = dout("gram_out", [192, 256])
    kss_out = dout("kss_out", [1, 192])
    qstats_out = dout("qstats_out", [128, 2, 2])
    vstats_out = dout("vstats_out", [128, 2, 2])

    with tile.TileContext(nc) as tc, ExitStack() as ctx:
        wpool = ctx.enter_context(tc.tile_pool(name="weights", bufs=1))
        xpool = ctx.enter_context(tc.tile_pool(name="xslab", bufs=2))
        bigpool = ctx.enter_context(tc.tile_pool(name="big", bufs=1))
        midpool = ctx.enter_context(tc.tile_pool(name="mid", bufs=2))
        smpool = ctx.enter_context(tc.tile_pool(name="small", bufs=4))
        statpool = ctx.enter_context(tc.tile_pool(name="stats", bufs=1))
        pspool = ctx.enter_context(tc.tile_pool(name="ps", bufs=5, space="PSUM"))
        pspers = ctx.enter_context(tc.tile_pool(name="pspers", bufs=1, space="PSUM"))

        def load1(ap_in, shape, dt=F8, eng=None):
            t = wpool.tile(shape, dt, tag=ap_in.tensor.name)
            (eng or nc.sync).dma_start(out=t[:ap_in.shape[0]], in_=ap_in[:])
            return t

        # phase-A-critical constants on the sync queue; the rest via gpsimd queue
        qwT_s = load1(qwT, [128, 6, 256])
        x1b_s = load1(x1_bias, [128, 6], F32)
        g = nc.gpsimd
        qdw_s = load1(qdw_wT, [128, 9 * 6, 128], BF16, eng=g)
        cw_s = load1(cw, [128, 7, 3, 256], eng=g)
        kvdw_s = load1(kvdw_wT, [128, 9 * 6, 64], BF16, eng=g)
        nkw_s = load1(nkw, [128, 2, 2, 192], eng=g)
        nvw_s = load1(nvw, [128, 2, 2, 256], eng=g)
        id_s = load1(ident, [128, 128], eng=g)
        ones_s = load1(ones_col, [128, 1], BF16, eng=g)
        qkvb_s = load1(qkv_bias, [128, 6], F32, eng=g)
        kvb_s = load1(kv_bias, [128, 3], F32, eng=g)
        dwb_s = load1(kvdw_bias, [128, 3], F32, eng=g)
        nvb_s = load1(newv_bias, [128, 2], F32, eng=g)
        nkb_bc = wpool.tile([128, 192], F32, tag="nkb_bc")
        nc.gpsimd.dma_start(out=nkb_bc[:], in_=_bcast(newk_b_row[0:1, :], 128))

        gramA = pspers.tile([128, 256], F32)
        gramB = pspers.tile([64, 256], F32)
        kss_ps = pspers.tile([1, 192], F32)

        qstats = statpool.tile([128, 2, NSLAB * (NS // 512), 6], F32)
        vstats = statpool.tile([128, 2, NSLAB * (NS // 512), 6], F32)

        n128_total = NSLAB * N128

        for s in range(NSLAB):
            xsl = xpool.tile([128, 2, XCOLS], F8, tag="xsl")
            mc = xpool.tile([128, 3, MCW], F8, tag="mc")
            off = s * S * WP
            nc.sync.dma_start(out=xsl[:], in_=xc[:, :, off:off + XCOLS])
            nc.sync.dma_start(out=mc[:, 0, 1:1 + MCOLS], in_=xm_lo[:, off:off + MCOLS])
            nc.sync.dma_start(out=mc[:, 1, 1:1 + MCOLS], in_=xm_d1[:, off:off + MCOLS])
            nc.sync.dma_start(out=mc[:, 2, 1:1 + MCOLS], in_=xm_d2[:, off:off + MCOLS])
            for pl in range(3):
                nc.vector.memset(mc[:, pl, 0:1], 0.0)
                nc.vector.memset(mc[:, pl, MCW - 1:MCW], 0.0)
            mtile = xpool.tile([128, XCOLS], BF16, tag="mtile")
            nc.sync.dma_start(out=mtile[:], in_=_bcast(mask_rc[0:1, off:off + XCOLS], 128))

            # ---- Phase A: x1 = 1x1(x): one fp8 DoubleRow pass per (pb, c0)
            x1 = bigpool.tile([128, 6, XCOLS], BF16, tag="x1")
            for pb in range(6):
                for c0 in range(0, XCOLS, 512):
                    cs = min(512, XCOLS - c0)
                    ps = pspool.tile([128, 512], F32, tag="ps", name=f"psA{s}_{pb}_{c0}")
                    nc.tensor.matmul(
                        ps[:, :cs],
                        qwT_s[:, pb, :].rearrange("p (m two) -> p two m", two=2),
                        xsl[:, :, c0:c0 + cs], start=True, stop=True, perf_mode=DRI)
                    nc.vector.scalar_tensor_tensor(
                        out=x1[:, pb, c0:c0 + cs], in0=ps[:, :cs],
                        scalar=x1b_s[:, pb:pb + 1], in1=mtile[:, c0:c0 + cs],
                        op0=mybir.AluOpType.add, op1=mybir.AluOpType.mult)

            # ---- Phase C: kv1 = 3x3(xm): 7 fp8 DoubleRow tap-pair passes
            # pair rhs (base offset into mc, plane delta):
            CPASS = [(0, 1), (2, WP - 2), (WP + 1, 1), (2 * WP, 1),
                     (2 * WP + 2, MCW - 1),
                     (MCW, 1), (MCW + 2, MCW + 2 * WP - 2)]
            kv1 = bigpool.tile([128, 3, XCOLS], BF16, tag="kv1")
            for c0 in range(0, XCOLS, 512):
                cs = min(512, XCOLS - c0)
                for pb in range(3):
                    ps = pspool.tile([128, 512], F32, tag="ps", name=f"psC{s}_{pb}_{c0}")
                    for i, (base, delta) in enumerate(CPASS):
                        nc.tensor.matmul(
                            ps[:, :cs],
                            cw_s[:, i, pb, :].rearrange("p (m two) -> p two m", two=2),
                            _dr3(mc[:, 0, :], base + c0, delta, cs),
                            start=(i == 0), stop=(i == 6), perf_mode=DRI)
                    nc.vector.scalar_tensor_tensor(
                        out=kv1[:, pb, c0:c0 + cs], in0=ps[:, :cs],
                        scalar=kvb_s[:, pb:pb + 1], in1=mtile[:, c0:c0 + cs],
                        op0=mybir.AluOpType.add, op1=mybir.AluOpType.mult)

            # ---- Phase B: qkv = qdw(x1): pblock pairs, 4 concurrent 64x64 blocks
            qkv = bigpool.tile([128, 6, NS], F8, tag="qkv")
            x1v = [x1[:, pb, :].rearrange("p (r c) -> p r c", c=WP) for pb in range(6)]
            for sset in range(3):
                for ic in range(S // 4):
                    r0 = ic * 4
                    pse = pspool.tile([128, 512], F32, tag="ps", name=f"psBe{s}_{sset}_{ic}")
                    pso = pspool.tile([128, 512], F32, tag="ps", name=f"psBo{s}_{sset}_{ic}")
                    for ti, (dy, dx) in enumerate(TAPS):
                        for j in (0, 1):
                            pb = 2 * sset + j
                            pst = pse if j == 0 else pso
                            for h in (0, 1):
                                qb = 64 * h if j == 0 else 64 * (1 - h)
                                rhs = x1v[pb][64 * h:64 * h + 64,
                                              r0 + 1 + dy:r0 + 5 + dy, 1 + dx:1 + dx + 128]
                                nc.tensor.matmul(
                                    pst[qb:qb + 64, :],
                                    qdw_s[64 * h:64 * h + 64, 6 * ti + pb, qb:qb + 64],
                                    rhs, start=(ti == 0), stop=(ti == 8),
                                    tile_position=(64 * h, qb), skip_group_check=True)
                    for j in (0, 1):
                        pb = 2 * sset + j
                        nc.scalar.activation(out=qkv[:, pb, r0 * 128:(r0 + 4) * 128],
                                             in_=(pse if j == 0 else pso)[:, :],
                                             func=mybir.ActivationFunctionType.Identity,
                                             bias=qkvb_s[:, pb:pb + 1], scale=SQKV)

            # ---- Phase D: kvdw via 6 concurrent 64x64 diag blocks
            dw = bigpool.tile([128, 3, NS], F8, tag="dw")
            kv1v = [kv1[:, pb, :].rearrange("p (r c) -> p r c", c=WP) for pb in range(3)]
            for ic in range(S // 4):
                r0 = ic * 4
                pss = [pspool.tile([128, 512], F32, tag="ps", name=f"psD{s}_{ic}_{i}")
                       for i in range(3)]
                for ti, (dy, dx) in enumerate(TAPS):
                    for blk, (ipb, ib, opb, ob) in enumerate(DWBLK):
                        rhs = kv1v[ipb][ib:ib + 64,
                                        r0 + 1 + dy:r0 + 5 + dy, 1 + dx:1 + dx + 128]
                        nc.tensor.matmul(pss[opb][ob:ob + 64, :],
                                         kvdw_s[ib:ib + 64, 6 * ti + blk, :],
                                         rhs, start=(ti == 0), stop=(ti == 8),
                                         tile_position=(ib, ob), skip_group_check=True)
                for pb in range(3):
                    nc.scalar.activation(out=dw[:, pb, r0 * 128:(r0 + 4) * 128],
                                         in_=pss[pb][:, :],
                                         func=mybir.ActivationFunctionType.Identity,
                                         bias=dwb_s[:, pb:pb + 1], scale=SDW8)

            # ---- Phase E: v = newv(v_cc): 2 fp8 DoubleRow passes per (c0, mb)
            vt = midpool.tile([128, 2, NS], F8, tag="vt")
            for ic in range(NS // 512):
                c0 = ic * 512
                for mb in range(2):
                    msz = 128 if mb == 0 else 64
                    ps = pspool.tile([128, 512], F32, tag="ps", name=f"psE{s}_{ic}_{mb}")
                    nc.tensor.matmul(
                        ps[:, :],
                        nvw_s[:, 0, mb, :].rearrange("p (m two) -> p two m", two=2),
                        qkv[:, 4:6, c0:c0 + 512], start=True, stop=False, perf_mode=DRI)
                    nc.tensor.matmul(
                        ps[:, :],
                        nvw_s[:, 1, mb, :].rearrange("p (m two) -> p two m", two=2),
                        dw[:, 1:3, c0:c0 + 512], start=False, stop=True, perf_mode=DRI)
                    nc.scalar.activation(out=vt[:msz, mb, c0:c0 + 512], in_=ps[:msz, :],
                                         func=mybir.ActivationFunctionType.Identity,
                                         bias=nvb_s[:msz, mb:mb + 1], scale=SV8 / SKE)
            nc.sync.dma_start(out=v_out[:, 0, s * NS:(s + 1) * NS], in_=vt[:, 0, :])
            nc.sync.dma_start(out=v_out[0:64, 1, s * NS:(s + 1) * NS], in_=vt[:64, 1, :])
            for sub in range(NS // 512):
                si = s * (NS // 512) + sub
                sl = slice(sub * 512, (sub + 1) * 512)
                nc.vector.bn_stats(out=vstats[:, 0, si, :], in_=vt[:, 0, sl])
                nc.vector.bn_stats(out=vstats[:64, 1, si, :], in_=vt[:64, 1, sl])
                nc.vector.bn_stats(out=qstats[:, 0, si, :], in_=qkv[:, 0, sl])
                nc.vector.bn_stats(out=qstats[:, 1, si, :], in_=qkv[:, 1, sl])

            # ---- Phase F: per 256-px group: kT pair, qT pair, DoubleRow gram, kss
            for gi in range(N128 // 2):
                gidx = s * (N128 // 2) + gi
                kT2 = smpool.tile([128, 2, 192], F8, tag="kT2")
                # fp8 transpose requires output element step 2: interleaved psum
                qps = pspool.tile([128, 2, 512], F8, tag="ps", name=f"qps{s}_{gi}")
                qps_f = qps[:, 0, :]
                pstr = list(qps_f.ap[0])
                for ci in (0, 1):
                    c0 = (2 * gi + ci) * 128
                    kps = pspool.tile([128, 192], F32, tag="ps", name=f"kps{s}_{gi}_{ci}")
                    nc.tensor.matmul(kps[:, :], qkv[:, 2:4, c0:c0 + 128],
                                     nkw_s[:, 0, :, :], start=True, stop=False,
                                     perf_mode=DR)
                    nc.tensor.matmul(kps[:, :], dw[:, 0:2, c0:c0 + 128],
                                     nkw_s[:, 1, :, :], start=False, stop=True,
                                     perf_mode=DR)
                    nc.vector.scalar_tensor_tensor(
                        out=kT2[:, ci, :], in0=kps[:], scalar=SK8 / SKE, in1=nkb_bc[:],
                        op0=mybir.AluOpType.mult, op1=mybir.AluOpType.add)
                    for ch in (0, 1):
                        o2 = bass.AP(tensor=qps_f.tensor,
                                     offset=qps_f.offset + ci * 512 + ch * 256,
                                     ap=[pstr, [2, 128]])
                        nc.tensor.transpose(o2, qkv[:, ch, c0:c0 + 128], id_s[:, :])
                    ksq = smpool.tile([128, 192], BF16, tag="ksq")
                    nc.vector.tensor_mul(ksq[:], kT2[:, ci, :], kT2[:, ci, :])
                    nc.tensor.matmul(kss_ps[:, :], ones_s[:, :], ksq[:],
                                     start=(gidx == 0 and ci == 0),
                                     stop=(gidx == n128_total // 2 - 1 and ci == 1))
                qT2 = smpool.tile([128, 2, 256], F8, tag="qT2")
                qps_v = bass.AP(tensor=qps_f.tensor, offset=qps_f.offset,
                                ap=[pstr, [512, 2], [256, 2], [2, 128]])
                qT2_v = qT2[:].rearrange("p c (h n) -> p c h n", h=2)
                nc.scalar.copy(out=qT2_v, in_=qps_v)
                nc.tensor.matmul(gramA[:, :], kT2[:, :, 0:128], qT2[:],
                                 start=(gidx == 0), stop=(gidx == n128_total // 2 - 1),
                                 perf_mode=DR)
                nc.tensor.matmul(gramB[:, :], kT2[:, :, 128:192], qT2[:],
                                 start=(gidx == 0), stop=(gidx == n128_total // 2 - 1),
                                 perf_mode=DR)

        qmv = statpool.tile([128, 2, 2], F32)
        vmv = statpool.tile([128, 2, 2], F32)
        nc.vector.memset(qmv[:], 0.0)
        nc.vector.memset(vmv[:], 0.0)
        nc.vector.bn_aggr(out=qmv[:, 0, :], in_=qstats[:, 0, :, :])
        nc.vector.bn_aggr(out=qmv[:, 1, :], in_=qstats[:, 1, :, :])
        nc.vector.bn_aggr(out=vmv[:, 0, :], in_=vstats[:, 0, :, :])
        nc.vector.bn_aggr(out=vmv[:64, 1, :], in_=vstats[:64, 1, :, :])
        nc.sync.dma_start(out=qstats_out[:], in_=qmv[:])
        nc.sync.dma_start(out=vstats_out[:], in_=vmv[:])
        gA = statpool.tile([128, 256], F32)
        gB = statpool.tile([64, 256], F32)
        kssb = statpool.tile([1, 192], F32)
        nc.scalar.copy(out=gA[:], in_=gramA[:])
        nc.scalar.copy(out=gB[:], in_=gramB[:])
        nc.scalar.copy(out=kssb[:], in_=kss_ps[:])
        nc.sync.dma_start(out=gram_out[0:128, :], in_=gA[:])
        nc.sync.dma_start(out=gram_out[128:192, :], in_=gB[:])
        nc.sync.dma_start(out=kss_out[:], in_=kssb[:])

    nc.compile()
    return nc


def build_l2(R=64, W=128):
    NS = R * W
    nc = bacc.Bacc("TRN2", target_bir_lowering=False, debug=False, num_devices=8)
    v_in = nc.dram_tensor("v_in", [128, 2 * NS], F8, kind="ExternalInput").ap()
    awT = nc.dram_tensor("awT", [128, 2, 256], F8, kind="ExternalInput").ap()
    pbias = nc.dram_tensor("pbias", [128, 2], F32, kind="ExternalInput").ap()
    sab = nc.dram_tensor("sab", [128, 2], F32, kind="ExternalInput").ap()  # 1/SA
    pbraw = nc.dram_tensor("pbraw", [128, 2], F32, kind="ExternalInput").ap()
    out = nc.dram_tensor("out", [128, 2, NS], BF16, kind="ExternalOutput").ap()

    with tile.TileContext(nc) as tc, ExitStack() as ctx:
        wpool = ctx.enter_context(tc.tile_pool(name="w", bufs=1))
        vpool = ctx.enter_context(tc.tile_pool(name="v", bufs=1))
        opool = ctx.enter_context(tc.tile_pool(name="o", bufs=1))
        pspool = ctx.enter_context(tc.tile_pool(name="ps", bufs=7, space="PSUM"))

        aw = wpool.tile([128, 2, 256], F8)
        nc.sync.dma_start(out=aw[:], in_=awT[:])
        pb = wpool.tile([128, 2], F32)
        nc.sync.dma_start(out=pb[:], in_=pbias[:])
        sav = wpool.tile([128, 2], F32)
        nc.sync.dma_start(out=sav[:], in_=sab[:])
        pbr = wpool.tile([128, 2], F32)
        nc.sync.dma_start(out=pbr[:], in_=pbraw[:])
        CH = 2048
        vts = []
        for ci in range(NS // CH):
            vtc = vpool.tile([128, 2, CH], F8, name=f"vt{ci}")
            q = (nc.sync, nc.gpsimd)[ci % 2]
            q.dma_start(out=vtc[:], in_=_dr3(v_in[:], ci * CH, NS, CH))
            vts.append(vtc)
        ot = opool.tile([128, 2, NS], BF16)

        # mb-outer: consecutive matmuls share the stationary operand
        for mb in range(2):
            msz = 128 if mb == 0 else 64
            for ic in range(NS // 512):
                c0 = ic * 512
                vtc = vts[c0 // CH]
                cc = c0 % CH
                ps = pspool.tile([128, 512], F32, tag="ps", name=f"ps{mb}_{ic}")
                nc.tensor.matmul(
                    ps[:, :],
                    aw[:, mb, :].rearrange("p (m two) -> p two m", two=2),
                    vtc[:, :, cc:cc + 512], start=True, stop=True, perf_mode=DRI)
                if ic % 2 == 0:
                    nc.vector.tensor_scalar(
                        out=ot[:msz, mb, c0:c0 + 512], in0=ps[:msz, :],
                        scalar1=pb[:msz, mb:mb + 1], scalar2=sav[:msz, mb:mb + 1],
                        op0=mybir.AluOpType.add, op1=mybir.AluOpType.mult)
                else:
                    nc.scalar.activation(
                        out=ot[:msz, mb, c0:c0 + 512], in_=ps[:msz, :],
                        func=mybir.ActivationFunctionType.Identity,
                        bias=pbr[:msz, mb:mb + 1], scale=sav[:msz, mb:mb + 1])
                if ic % 8 == 7:
                    h0 = (ic // 8) * 8 * 512
                    nc.sync.dma_start(out=out[:, mb, h0:h0 + 4096],
                                      in_=ot[:, mb, h0:h0 + 4096])
    nc.compile()
    return nc


WP = 130


def _dw_channel_at(pb, p):
    """kv channel (0..383) stored at partition p of dw pblock pb."""
    for (ipb, ib, opb, ob) in DWBLK:
        if opb == pb and ob <= p < ob + 64:
            return 128 * ipb + ib + (p - ob)
    return None


def prep_weights(w):
    """w: dict of reference weights (numpy f32). Returns dict of L1 input arrays."""
    out = {}
    qw = w["q_w"][:, :, 0, 0]          # (576, 192)
    qwT = np.zeros((128, 2, 768), np.float32)
    for (pb, h, x1b, qb, ch0, nch) in qkv_halves():
        win = 128 * pb + 64 * h
        qwT[0:128, 0, win:win + nch] = qw.T[0:128, ch0:ch0 + nch]
        qwT[0:64, 1, win:win + nch] = qw.T[128:192, ch0:ch0 + nch]
    qwTi = np.zeros((128, 6, 256), np.float32)
    for pb in range(6):
        qwTi[:, pb] = dri_pack(qwT[:, :, 128 * pb:128 * pb + 128])
    out["qwT"] = qf8(qwTi, SW)

    qdw = w["qdw_w"]                   # (576, 3, 3, 3) out, in-per-group, ky, kx
    qdwT = np.zeros((128, 54, 128), np.float32)
    for dy in (-1, 0, 1):
        for dx in (-1, 0, 1):
            ti = 3 * dy + dx + 4
            for (pb, h, x1b, qb, ch0, nch) in qkv_halves():
                for gl in range(nch // 3):
                    for i in range(3):
                        for j in range(3):
                            qdwT[x1b + 3 * gl + i, 6 * ti + pb, qb + 3 * gl + j] = \
                                qdw[ch0 + 3 * gl + j, i, dy + 1, dx + 1]
    out["qdw_wT"] = qdwT.astype(bf16)

    kvw = w["kv_w"]                    # (384, 192, 3, 3)
    # lo taps (128-ch rows) and packed hi taps, as DoubleRow pairs
    lo = np.zeros((9, 128, 384), np.float32)
    for dy in (-1, 0, 1):
        for dx in (-1, 0, 1):
            ti = 3 * dy + dx + 4
            lo[ti] = kvw[:, 0:128, dy + 1, dx + 1].T
    hi7 = np.zeros((128, 384), np.float32)
    hi7[0:64] = kvw[:, :, 2, 1].T[128:192]           # tap (1, 0) hi block
    kvp = np.zeros((4, 128, 384), np.float32)
    for j, dx in enumerate((-1, 0, 1)):
        kvp[j, 0:64] = kvw[:, :, 0, dx + 1].T[128:192]    # (-1, dx)
        kvp[j, 64:128] = kvw[:, :, 1, dx + 1].T[128:192]  # (0, dx)
    kvp[3, 0:64] = kvw[:, :, 2, 0].T[128:192]     # (1, -1)
    kvp[3, 64:128] = kvw[:, :, 2, 2].T[128:192]   # (1, +1)
    cwa = np.zeros((128, 7, 2, 384), np.float32)
    for i, (a, b) in enumerate([(0, 1), (2, 3), (4, 5), (6, 7)]):
        cwa[:, i, 0] = lo[a]
        cwa[:, i, 1] = lo[b]
    cwa[:, 4, 0] = lo[8]
    cwa[:, 4, 1] = hi7
    cwa[:, 5, 0] = kvp[0]
    cwa[:, 5, 1] = kvp[1]
    cwa[:, 6, 0] = kvp[2]
    cwa[:, 6, 1] = kvp[3]
    cwi = np.zeros((128, 7, 3, 256), np.float32)
    for i in range(7):
        for pb in range(3):
            cwi[:, i, pb] = dri_pack(cwa[:, i, :, 128 * pb:128 * pb + 128])
    out["cw"] = qf8(cwi, SW)

    kvdw = w["kvdw_w"][:, 0]           # (384, 3, 3)
    dwT = np.zeros((128, 54, 64), np.float32)
    for dy in (-1, 0, 1):
        for dx in (-1, 0, 1):
            ti = 3 * dy + dx + 4
            d = kvdw[:, dy + 1, dx + 1]
            for blk, (ipb, ib, opb, ob) in enumerate(DWBLK):
                ch0 = 128 * ipb + ib
                dwT[ib:ib + 64, 6 * ti + blk, :] = np.diag(d[ch0:ch0 + 64])
    out["kvdw_wT"] = dwT.astype(bf16)

    # newk: pass0 = (qkv pb2, qkv pb3) @ SKE/SQKV; pass1 = (dw pb0, dw pb1-hi) @ SKE/SDW8
    nk = w["newk_w"][:, :, 0, 0]       # (192, 384): in = [k(192) | k_mask(192)]
    nkm = np.zeros((128, 2, 2, 192), np.float32)
    for pl, pb in enumerate((2, 3)):
        for p in range(128):
            ch = qkv_channel_at(pb, p)
            if ch is not None:
                nkm[p, 0, pl] = nk[:, ch - 192]          # k part: qkv ch 192-383
    for pl, pb in enumerate((0, 1)):
        for p in range(128):
            ch = _dw_channel_at(pb, p)
            if ch is not None and ch < 192:
                nkm[p, 1, pl] = nk[:, 192 + ch]          # k_mask: dw ch 0-191
    nkm[:, 0] *= SKE / SQKV / SW
    nkm[:, 1] *= SKE / SDW8 / SW
    out["nkw"] = qf8(nkm, SW)

    nv = w["newv_w"][:, :, 0, 0]       # (192, 384): in = [v(192) | v_mask(192)]
    nvm = np.zeros((128, 2, 2, 192), np.float32)
    for pl, pb in enumerate((4, 5)):
        for p in range(128):
            ch = qkv_channel_at(pb, p)
            if ch is not None:
                nvm[p, 0, pl] = nv[:, ch - 384]          # v part: qkv ch 384-575
    for pl, pb in enumerate((1, 2)):
        for p in range(128):
            ch = _dw_channel_at(pb, p)
            if ch is not None and ch >= 192:
                nvm[p, 1, pl] = nv[:, ch]                # v_mask: dw ch 192-383
    nvm[:, 0] *= SKE / SQKV / SW
    nvm[:, 1] *= SKE / SDW8 / SW
    nvp = np.zeros((128, 2, 2, 2, 128), np.float32)
    nvp[:, :, :, 0, :] = nvm[:, :, :, 0:128]
    nvp[:, :, :, 1, 0:64] = nvm[:, :, :, 128:192]
    nvwi = np.zeros((128, 2, 2, 256), np.float32)
    for ps_ in range(2):
        for mb in range(2):
            nvwi[:, ps_, mb] = dri_pack(nvp[:, ps_, :, mb, :])
    out["nvw"] = qf8(nvwi, SW)

    out["ident"] = np.eye(128, dtype=e4m3)
    out["ones_col"] = np.ones((128, 1), dtype=bf16)

    x1b = np.zeros((128, 6), np.float32)
    qkvb = np.zeros((128, 6), np.float32)
    for (pb, h, x1b_base, qb, ch0, nch) in qkv_halves():
        x1b[x1b_base:x1b_base + nch, pb] = w["q_b"][ch0:ch0 + nch]
        qkvb[qb:qb + nch, pb] = w["qdw_b"][ch0:ch0 + nch]
    out["x1_bias"] = x1b * (SXI * SW)
    out["qkv_bias"] = qkvb * SQKV

    kvb = np.zeros((128, 3), np.float32)
    kvb[:, 0] = w["kv_b"][0:128]
    kvb[0:64, 1] = w["kv_b"][128:192]
    kvb[64:128, 1] = w["kv_b"][192:256]
    kvb[:, 2] = w["kv_b"][256:384]
    out["kv_bias"] = kvb * (SXI * SW)
    dwb = np.zeros((128, 3), np.float32)
    for (ipb, ib, opb, ob) in DWBLK:
        dwb[ob:ob + 64, opb] = w["kvdw_b"][128 * ipb + ib:128 * ipb + ib + 64]
    out["kvdw_bias"] = dwb * SDW8
    out["newk_b_row"] = w["newk_b"][None, :].astype(np.float32) * SK8
    nvb = np.zeros((128, 2), np.float32)
    nvb[:, 0] = w["newv_b"][0:128]
    nvb[0:64, 1] = w["newv_b"][128:192]
    out["newv_bias"] = nvb * SV8
    return out


def prep_masks(R, H, half):
    m = np.zeros((R + 2, WP), np.float32)
    for r in range(R + 2):
        g = half * R + (r - 1)
        if 0 <= g < H:
            m[r, 1:129] = MINV
    return m.reshape(1, -1)


def prep_core(x, xm, b, half, R, H):
    xp = np.zeros((192, R + 2, WP), np.float32)
    mp = np.zeros((192, R + 4, WP), np.float32)
    for r in range(R + 2):
        g = half * R + (r - 1)
        if 0 <= g < H:
            xp[:, r, 1:129] = x[b, :, g, :]
    for r in range(R + 4):
        g = half * R + (r - 2)
        if 0 <= g < H:
            mp[:, r, 1:129] = xm[b, :, g, :]
    xp = xp.reshape(192, -1)
    xcb = np.zeros((128, 2, xp.shape[1]), np.float32)
    xcb[:, 0] = xp[0:128]
    xcb[0:64, 1] = xp[128:192]
    mp = mp.reshape(192, -1)
    L = mp.shape[1]
    hi = mp[128:192]
    d1 = np.zeros((128, L), np.float32)
    d2 = np.zeros((128, L), np.float32)
    d1[0:64] = hi
    d1[64:128, :L - 130] = hi[:, 130:]
    d2[0:64] = hi
    d2[64:128, :L - 2] = hi[:, 2:]
    return {
        "xc": qf8(xcb, SXI),
        "xm_lo": qf8(mp[0:128], SXI),
        "xm_d1": qf8(d1, SXI), "xm_d2": qf8(d2, SXI),
        "mask_rc": prep_masks(R, H, half).astype(bf16),
    }


def _q_maps():
    """q channel c (0..191) -> (pblock 0/1, partition)."""
    part = np.zeros(192, np.int64)
    pblk = np.zeros(192, np.int64)
    for (pb, h, x1b, qb, ch0, nch) in qkv_halves():
        if pb >= 2:
            continue
        for i in range(nch):
            pblk[ch0 + i] = pb
            part[ch0 + i] = qb + i
    return pblk, part


def _ss_from_qstats(stats, n_half):
    pblk, part = _q_maps()
    mv = stats.astype(np.float64)
    return (mv[part, pblk, 1] + mv[part, pblk, 0] ** 2) * n_half


def _ss_from_vstats(stats, n_half):
    ss = np.zeros(192, np.float64)
    mv = stats.astype(np.float64)
    ss[0:128] = (mv[0:128, 0, 1] + mv[0:128, 0, 0] ** 2) * n_half
    ss[128:192] = (mv[0:64, 1, 1] + mv[0:64, 1, 0] ** 2) * n_half
    return ss


def glue(res0, res1, temperature, proj_w, proj_b, n_half):
    """Combine two half-core L1 results -> L2 inputs (awT fp8, pbias, sa)."""
    G = res0["gram_out"].astype(np.float64) + res1["gram_out"].astype(np.float64)
    pblk, part = _q_maps()
    qcol = pblk * 128 + part
    G = G[:, qcol]                              # (d, c): sum_n k[d,n] q[c,n]
    qss = _ss_from_qstats(res0["qstats_out"], n_half) + _ss_from_qstats(res1["qstats_out"], n_half)
    vss = _ss_from_vstats(res0["vstats_out"], n_half) + _ss_from_vstats(res1["vstats_out"], n_half)
    kss = (res0["kss_out"].astype(np.float64) + res1["kss_out"].astype(np.float64))[0]
    qn = np.maximum(np.sqrt(qss), 1e-12)
    kn = np.maximum(np.sqrt(kss), 1e-12)
    vn = np.maximum(np.sqrt(vss), 1e-12)
    A = G.T / (qn[:, None] * kn[None, :])      # (c, d)
    M = np.zeros((192, 192), np.float64)
    t = np.asarray(temperature).reshape(-1)
    for h in range(8):
        sl = slice(24 * h, 24 * h + 24)
        a = A[sl, sl] * t[h]
        a = a - a.max(axis=-1, keepdims=True)
        e = np.exp(a)
        sm = e / e.sum(axis=-1, keepdims=True)
        M[sl, sl] = sm / vn[None, sl]
    At = proj_w[:, :, 0, 0].astype(np.float64) @ M   # (out-ch o, d)
    SA = 2.0 ** np.floor(np.log2(128.0 / max(np.abs(At).max(), 1e-30)))
    awT = np.zeros((128, 2, 192), np.float32)
    awT[:, 0, :] = At.T[0:128]
    awT[0:64, 1, :] = At.T[128:192]
    awp = np.zeros((128, 2, 2, 128), np.float32)
    awp[:, :, 0, :] = awT[:, :, 0:128]
    awp[:, :, 1, 0:64] = awT[:, :, 128:192]
    awTi = np.zeros((128, 2, 256), np.float32)
    for mb in range(2):
        awTi[:, mb] = dri_pack(awp[:, :, mb, :])
    pbias = np.zeros((128, 2), np.float32)
    pbias[:, 0] = proj_b[0:128]
    pbias[0:64, 1] = proj_b[128:192]
    return {"awT": qf8(awTi, SA), "pbias": pbias * SA, "pbraw": pbias,
            "sab": np.full((128, 2), 1.0 / SA, np.float32)}


def _prep_vin(v_out):
    """L1 v_out [128, 2, NS] -> L2 v_in [128, 2*NS], pad rows zeroed."""
    vv = np.array(v_out)
    vv[64:128, 1, :] = np.zeros(1, e4m3)
    return vv.reshape(128, -1)


# ---------------- driver: kernel(**inputs) ----------------
from concourse.bass_utils import run_bass_kernel_spmd

R_FULL, H_FULL, B_FULL = 64, 128, 4
_NC1 = None
_NC2 = None


def _get_progs():
    global _NC1, _NC2
    if _NC1 is None:
        _NC1 = build_l1(R=R_FULL, S=16)
        _NC2 = build_l2(R=R_FULL)
    return _NC1, _NC2


def kernel(**inputs):
    inputs = {k: np.asarray(v) for k, v in inputs.items()}
    x, xm = inputs["x"], inputs["x_mask"]
    nc1, nc2 = _get_progs()
    wprep = prep_weights(inputs)
    in_maps = []
    for core in range(8):
        b, half = core // 2, core % 2
        m = dict(wprep)
        m.update(prep_core(x, xm, b, half, R_FULL, H_FULL))
        in_maps.append(m)
    res1 = run_bass_kernel_spmd(nc1, in_maps, list(range(8))).results

    n_half = R_FULL * 128
    in_maps2 = []
    for core in range(8):
        b, half = core // 2, core % 2
        if half == 0:
            l2c = glue(res1[2 * b], res1[2 * b + 1], inputs["temperature"],
                       inputs["proj_w"], inputs["proj_b"], n_half)
        m = dict(l2c)
        m["v_in"] = _prep_vin(res1[core]["v_out"])
        in_maps2.append(m)
    res2 = run_bass_kernel_spmd(nc2, in_maps2, list(range(8))).results

    out = np.empty((B_FULL, 192, H_FULL, 128), np.float32)
    for core in range(8):
        b, half = core // 2, core % 2
        o = np.asarray(res2[core]["out"]).astype(np.float32)   # [128, 2, NS] bf16
        sl = out[b, :, half * R_FULL:(half + 1) * R_FULL, :]
        sl[0:128] = o[:, 0, :].reshape(128, R_FULL, 128)
        sl[128:192] = o[0:64, 1, :].reshape(64, R_FULL, 128)
    return out


# revision 15
# speedup vs baseline: 1.7204x; 1.0692x over previous
"""Trainium2 Bass kernel for nn_Cross_Attention (sparse_attention, 8 cores).

fp8(e4m3)+DoubleRow version: phases A/C/E/F and L2 run fp8 DoubleRow matmuls
(two 128-deep contraction chunks per pass at ~0.57 cyc/col); B/D keep bf16
quadrant matmuls but emit fp8. Power-of-2 scale bookkeeping throughout.
Shards 4 samples x 2 row-halves across 8 NeuronCores, glues partials on host.
"""
import sys
sys.path.insert(0, "/opt/trn_rl_repo")
import numpy as np
import ml_dtypes

import concourse.bass as bass
import concourse.tile as tile
from concourse import bacc, mybir
from contextlib import ExitStack


BF16 = mybir.dt.bfloat16
F8 = mybir.dt.float8e4
F32 = mybir.dt.float32
bf16 = ml_dtypes.bfloat16
e4m3 = ml_dtypes.float8_e4m3
DR = mybir.MatmulPerfMode.DoubleRow
DRI = mybir.MatmulPerfMode.DoubleRowSwInterleave

TAPS = [(dy, dx) for dy in (-1, 0, 1) for dx in (-1, 0, 1)]

# power-of-2 scales
SXI = 16.0        # x, x_mask fp8 pre-scale (2^4)
SW = 1024.0       # conv-weight fp8 pre-scale (2^10)
MINV = 1.0 / (SXI * SW)   # mask value de-scaling x1/kv1 back to true (2^-14)
SQKV = 512.0      # qkv fp8 storage scale (2^9)
SDW8 = 256.0      # dw fp8 storage scale (2^8)
SK8 = 512.0       # kT fp8 storage scale (2^9)
SV8 = 512.0       # v fp8 storage scale (2^9)
SKE = 2.0 ** 19   # newk/newv psum scale


def qf8(a, s):
    return np.clip(np.asarray(a, np.float32) * s, -240, 240).astype(e4m3)


def dri_pack(w2m):
    """[128, 2, M] -> [128, 2M] DoubleRowSwInterleave layout (pairs, reversed)."""
    a, b = w2m[:, 0], w2m[:, 1]
    inter = np.empty((w2m.shape[0], 2 * w2m.shape[2]), np.float32)
    inter[:, 0::2] = a[:, ::-1]
    inter[:, 1::2] = b[:, ::-1]
    return inter


def qkv_halves():
    """Per (pb, half): (x1_base, qkv_base, ch0, nch).  ch0 = qkv-global channel."""
    out = []
    for pb in range(6):
        P, odd = pb // 2, pb % 2
        for h in (0, 1):
            nch = 3 if (odd and h == 1) else 63
            ch0 = 3 * (64 * P + 42 * odd + 21 * h)
            x1b = 64 * h
            qb = 64 * h if not odd else 64 * (1 - h)
            out.append((pb, h, x1b, qb, ch0, nch))
    return out


def qkv_channel_at(pb, p):
    """qkv-global channel stored at partition p of qkv pblock pb, or None."""
    for (pb2, h, x1b, qb, ch0, nch) in qkv_halves():
        if pb2 == pb and qb <= p < qb + nch:
            return ch0 + (p - qb)
    return None


# newk/newv input chunk pairs (DoubleRow planes), see prep_weights
# kvdw 64-blocks: (in_pb, in_base, out_pb, out_base); dw pb1 halves swapped
DWBLK = [(0, 0, 0, 0), (0, 64, 0, 64), (1, 0, 1, 64),
         (1, 64, 1, 0), (2, 0, 2, 0), (2, 64, 2, 64)]


def _bcast(ap, p):
    return bass.AP(tensor=ap.tensor, offset=ap.offset, ap=[[0, p]] + list(ap.ap[1:]))


def _dr3(tile_ap, base, delta, cs):
    """[P, 2, cs] DoubleRow rhs view of a tile at element offset base,
    plane delta `delta`."""
    return bass.AP(tensor=tile_ap.tensor, offset=tile_ap.offset + base,
                   ap=[list(tile_ap.ap[0]), [delta, 2], [1, cs]])


def build_l1(R=64, W=128, S=16):
    assert W == 128 and R % S == 0 and S % 4 == 0
    WP = W + 2  # 130
    NSLAB = R // S
    NS = S * W
    XCOLS = (S + 2) * WP
    MCOLS = (S + 4) * WP
    MCW = MCOLS + 2
    N128 = NS // 128

    nc = bacc.Bacc("TRN2", target_bir_lowering=False, debug=False, num_devices=8)

    def din(name, shape, dt=F8):
        return nc.dram_tensor(name, shape, dt, kind="ExternalInput").ap()

    def dout(name, shape, dt=F32):
        return nc.dram_tensor(name, shape, dt, kind="ExternalOutput").ap()

    xc = din("xc", [128, 2, (R + 2) * WP])
    xm_lo = din("xm_lo", [128, (R + 4) * WP])
    xm_d1 = din("xm_d1", [128, (R + 4) * WP])
    xm_d2 = din("xm_d2", [128, (R + 4) * WP])
    qwT = din("qwT", [128, 6, 256])            # SwInterleave-packed per pblock
    qdw_wT = din("qdw_wT", [128, 9 * 6, 128], BF16)
    cw = din("cw", [128, 7, 3, 256])           # kv 3x3 tap-pair weights (interleaved)
    kvdw_wT = din("kvdw_wT", [128, 9 * 6, 64], BF16)
    nkw = din("nkw", [128, 2, 2, 192])
    nvw = din("nvw", [128, 2, 2, 256])
    ident = din("ident", [128, 128])
    ones_col = din("ones_col", [128, 1], BF16)
    x1_bias = din("x1_bias", [128, 6], F32)
    qkv_bias = din("qkv_bias", [128, 6], F32)
    kv_bias = din("kv_bias", [128, 3], F32)
    kvdw_bias = din("kvdw_bias", [128, 3], F32)
    newk_b_row = din("newk_b_row", [1, 192], F32)
    newv_bias = din("newv_bias", [128, 2], F32)
    mask_rc = din("mask_rc", [1, (R + 2) * WP], BF16)

    v_out = dout("v_out", [128, 2, R * W], F8)
    gram_out = dout("gram_out", [192, 256])
    kss_out = dout("kss_out", [1, 192])
    qstats_out = dout("qstats_out", [128, 2, 2])
    vstats_out = dout("vstats_out", [128, 2, 2])

    with tile.TileContext(nc) as tc, ExitStack() as ctx:
        wpool = ctx.enter_context(tc.tile_pool(name="weights", bufs=1))
        xpool = ctx.enter_context(tc.tile_pool(name="xslab", bufs=2))
        bigpool = ctx.enter_context(tc.tile_pool(name="big", bufs=1))
        midpool = ctx.enter_context(tc.tile_pool(name="mid", bufs=2))
        smpool = ctx.enter_context(tc.tile_pool(name="small", bufs=4))
        statpool = ctx.enter_context(tc.tile_pool(name="stats", bufs=1))
        pspool = ctx.enter_context(tc.tile_pool(name="ps", bufs=5, space="PSUM"))
        pspers = ctx.enter_context(tc.tile_pool(name="pspers", bufs=1, space="PSUM"))

        def load1(ap_in, shape, dt=F8, eng=None):
            t = wpool.tile(shape, dt, tag=ap_in.tensor.name)
            (eng or nc.sync).dma_start(out=t[:ap_in.shape[0]], in_=ap_in[:])
            return t

        # phase-A-critical constants on the sync queue; the rest via gpsimd queue
        qwT_s = load1(qwT, [128, 6, 256])
        x1b_s = load1(x1_bias, [128, 6], F32)
        g = nc.gpsimd
        qdw_s = load1(qdw_wT, [128, 9 * 6, 128], BF16, eng=g)
        cw_s = load1(cw, [128, 7, 3, 256], eng=g)
        kvdw_s = load1(kvdw_wT, [128, 9 * 6, 64], BF16, eng=g)
        nkw_s = load1(nkw, [128, 2, 2, 192], eng=g)
        nvw_s = load1(nvw, [128, 2, 2, 256], eng=g)
        id_s = load1(ident, [128, 128], eng=g)
        ones_s = load1(ones_col, [128, 1], BF16, eng=g)
        qkvb_s = load1(qkv_bias, [128, 6], F32, eng=g)
        kvb_s = load1(kv_bias, [128, 3], F32, eng=g)
        dwb_s = load1(kvdw_bias, [128, 3], F32, eng=g)
        nvb_s = load1(newv_bias, [128, 2], F32, eng=g)
        nkb_bc = wpool.tile([128, 192], F32, tag="nkb_bc")
        nc.gpsimd.dma_start(out=nkb_bc[:], in_=_bcast(newk_b_row[0:1, :], 128))

        gramA = pspers.tile([128, 256], F32)
        gramB = pspers.tile([64, 256], F32)
        kss_ps = pspers.tile([1, 192], F32)

        qstats = statpool.tile([128, 2, NSLAB * (NS // 512), 6], F32)
        vstats = statpool.tile([128, 2, NSLAB * (NS // 512), 6], F32)

        n128_total = NSLAB * N128

        for s in range(NSLAB):
            xsl = xpool.tile([128, 2, XCOLS], F8, tag="xsl")
            mc = xpool.tile([128, 3, MCW], F8, tag="mc")
            off = s * S * WP
            nc.sync.dma_start(out=xsl[:], in_=xc[:, :, off:off + XCOLS])
            nc.sync.dma_start(out=mc[:, 0, 1:1 + MCOLS], in_=xm_lo[:, off:off + MCOLS])
            nc.sync.dma_start(out=mc[:, 1, 1:1 + MCOLS], in_=xm_d1[:, off:off + MCOLS])
            nc.sync.dma_start(out=mc[:, 2, 1:1 + MCOLS], in_=xm_d2[:, off:off + MCOLS])
            for pl in range(3):
                nc.vector.memset(mc[:, pl, 0:1], 0.0)
                nc.vector.memset(mc[:, pl, MCW - 1:MCW], 0.0)
            mtile = xpool.tile([128, XCOLS], BF16, tag="mtile")
            nc.sync.dma_start(out=mtile[:], in_=_bcast(mask_rc[0:1, off:off + XCOLS], 128))

            # ---- Phase A: x1 = 1x1(x): one fp8 DoubleRow pass per (pb, c0)
            x1 = bigpool.tile([128, 6, XCOLS], BF16, tag="x1")
            for pb in range(6):
                for c0 in range(0, XCOLS, 512):
                    cs = min(512, XCOLS - c0)
                    ps = pspool.tile([128, 512], F32, tag="ps", name=f"psA{s}_{pb}_{c0}")
                    nc.tensor.matmul(
                        ps[:, :cs],
                        qwT_s[:, pb, :].rearrange("p (m two) -> p two m", two=2),
                        xsl[:, :, c0:c0 + cs], start=True, stop=True, perf_mode=DRI)
                    if (c0 // 512) % 2 == 0:
                        nc.vector.tensor_scalar_mul(
                            out=x1[:, pb, c0:c0 + cs], in0=ps[:, :cs], scalar1=MINV)
                    else:
                        nc.scalar.activation(
                            out=x1[:, pb, c0:c0 + cs], in_=ps[:, :cs],
                            func=mybir.ActivationFunctionType.Identity, scale=MINV)

            # ---- Phase C: kv1 = 3x3(xm): 7 fp8 DoubleRow tap-pair passes
            # pair rhs (base offset into mc, plane delta):
            CPASS = [(0, 1), (2, WP - 2), (WP + 1, 1), (2 * WP, 1),
                     (2 * WP + 2, MCW - 1),
                     (MCW, 1), (MCW + 2, MCW + 2 * WP - 2)]
            kv1 = bigpool.tile([128, 3, XCOLS], BF16, tag="kv1")
            for c0 in range(0, XCOLS, 512):
                cs = min(512, XCOLS - c0)
                for pb in range(3):
                    ps = pspool.tile([128, 512], F32, tag="ps", name=f"psC{s}_{pb}_{c0}")
                    for i, (base, delta) in enumerate(CPASS):
                        nc.tensor.matmul(
                            ps[:, :cs],
                            cw_s[:, i, pb, :].rearrange("p (m two) -> p two m", two=2),
                            _dr3(mc[:, 0, :], base + c0, delta, cs),
                            start=(i == 0), stop=(i == 6), perf_mode=DRI)
                    nc.vector.scalar_tensor_tensor(
                        out=kv1[:, pb, c0:c0 + cs], in0=ps[:, :cs],
                        scalar=kvb_s[:, pb:pb + 1], in1=mtile[:, c0:c0 + cs],
                        op0=mybir.AluOpType.add, op1=mybir.AluOpType.mult)

            # ---- Phase B: qkv = qdw(x1): pblock pairs, 4 concurrent 64x64 blocks
            qkv = bigpool.tile([128, 6, NS], F8, tag="qkv")
            x1v = [x1[:, pb, :].rearrange("p (r c) -> p r c", c=WP) for pb in range(6)]
            for sset in range(3):
                for ic in range(S // 4):
                    r0 = ic * 4
                    pse = pspool.tile([128, 512], F32, tag="ps", name=f"psBe{s}_{sset}_{ic}")
                    pso = pspool.tile([128, 512], F32, tag="ps", name=f"psBo{s}_{sset}_{ic}")
                    for ti, (dy, dx) in enumerate(TAPS):
                        for j in (0, 1):
                            pb = 2 * sset + j
                            pst = pse if j == 0 else pso
                            for h in (0, 1):
                                qb = 64 * h if j == 0 else 64 * (1 - h)
                                rhs = x1v[pb][64 * h:64 * h + 64,
                                              r0 + 1 + dy:r0 + 5 + dy, 1 + dx:1 + dx + 128]
                                nc.tensor.matmul(
                                    pst[qb:qb + 64, :],
                                    qdw_s[64 * h:64 * h + 64, 6 * ti + pb, qb:qb + 64],
                                    rhs, start=(ti == 0), stop=(ti == 8),
                                    tile_position=(64 * h, qb), skip_group_check=True)
                    for j in (0, 1):
                        pb = 2 * sset + j
                        nc.scalar.activation(out=qkv[:, pb, r0 * 128:(r0 + 4) * 128],
                                             in_=(pse if j == 0 else pso)[:, :],
                                             func=mybir.ActivationFunctionType.Identity,
                                             bias=qkvb_s[:, pb:pb + 1], scale=SQKV)

            # ---- Phase D: kvdw via 6 concurrent 64x64 diag blocks
            dw = bigpool.tile([128, 3, NS], F8, tag="dw")
            kv1v = [kv1[:, pb, :].rearrange("p (r c) -> p r c", c=WP) for pb in range(3)]
            for ic in range(S // 4):
                r0 = ic * 4
                pss = [pspool.tile([128, 512], F32, tag="ps", name=f"psD{s}_{ic}_{i}")
                       for i in range(3)]
                for ti, (dy, dx) in enumerate(TAPS):
                    for blk, (ipb, ib, opb, ob) in enumerate(DWBLK):
                        rhs = kv1v[ipb][ib:ib + 64,
                                        r0 + 1 + dy:r0 + 5 + dy, 1 + dx:1 + dx + 128]
                        nc.tensor.matmul(pss[opb][ob:ob + 64, :],
                                         kvdw_s[ib:ib + 64, 6 * ti + blk, :],
                                         rhs, start=(ti == 0), stop=(ti == 8),
                                         tile_position=(ib, ob), skip_group_check=True)
                for pb in range(3):
                    nc.scalar.activation(out=dw[:, pb, r0 * 128:(r0 + 4) * 128],
                                         in_=pss[pb][:, :],
                                         func=mybir.ActivationFunctionType.Identity,
                                         bias=dwb_s[:, pb:pb + 1], scale=SDW8)

            # ---- Phase E: v = newv(v_cc): 2 fp8 DoubleRow passes per (c0, mb)
            vt = midpool.tile([128, 2, NS], F8, tag="vt")
            for ic in range(NS // 512):
                c0 = ic * 512
                for mb in range(2):
                    msz = 128 if mb == 0 else 64
                    ps = pspool.tile([128, 512], F32, tag="ps", name=f"psE{s}_{ic}_{mb}")
                    nc.tensor.matmul(
                        ps[:, :],
                        nvw_s[:, 0, mb, :].rearrange("p (m two) -> p two m", two=2),
                        qkv[:, 4:6, c0:c0 + 512], start=True, stop=False, perf_mode=DRI)
                    nc.tensor.matmul(
                        ps[:, :],
                        nvw_s[:, 1, mb, :].rearrange("p (m two) -> p two m", two=2),
                        dw[:, 1:3, c0:c0 + 512], start=False, stop=True, perf_mode=DRI)
                    nc.scalar.activation(out=vt[:msz, mb, c0:c0 + 512], in_=ps[:msz, :],
                                         func=mybir.ActivationFunctionType.Identity,
                                         bias=nvb_s[:msz, mb:mb + 1], scale=SV8 / SKE)
            nc.sync.dma_start(out=v_out[:, 0, s * NS:(s + 1) * NS], in_=vt[:, 0, :])
            nc.sync.dma_start(out=v_out[0:64, 1, s * NS:(s + 1) * NS], in_=vt[:64, 1, :])
            for sub in range(NS // 512):
                si = s * (NS // 512) + sub
                sl = slice(sub * 512, (sub + 1) * 512)
                nc.vector.bn_stats(out=vstats[:, 0, si, :], in_=vt[:, 0, sl])
                nc.vector.bn_stats(out=vstats[:64, 1, si, :], in_=vt[:64, 1, sl])
                nc.vector.bn_stats(out=qstats[:, 0, si, :], in_=qkv[:, 0, sl])
                nc.vector.bn_stats(out=qstats[:, 1, si, :], in_=qkv[:, 1, sl])

            # ---- Phase F: per 256-px group: kT pair, qT pair, DoubleRow gram, kss
            for gi in range(N128 // 2):
                gidx = s * (N128 // 2) + gi
                kT2 = smpool.tile([128, 2, 192], F8, tag="kT2")
                # fp8 transpose requires output element step 2: interleaved psum
                qps = pspool.tile([128, 2, 512], F8, tag="ps", name=f"qps{s}_{gi}")
                qps_f = qps[:, 0, :]
                pstr = list(qps_f.ap[0])
                for ci in (0, 1):
                    c0 = (2 * gi + ci) * 128
                    kps = pspool.tile([128, 192], F32, tag="ps", name=f"kps{s}_{gi}_{ci}")
                    nc.tensor.matmul(kps[:, :], qkv[:, 2:4, c0:c0 + 128],
                                     nkw_s[:, 0, :, :], start=True, stop=False,
                                     perf_mode=DR)
                    nc.tensor.matmul(kps[:, :], dw[:, 0:2, c0:c0 + 128],
                                     nkw_s[:, 1, :, :], start=False, stop=True,
                                     perf_mode=DR)
                    nc.vector.scalar_tensor_tensor(
                        out=kT2[:, ci, :], in0=kps[:], scalar=SK8 / SKE, in1=nkb_bc[:],
                        op0=mybir.AluOpType.mult, op1=mybir.AluOpType.add)
                    for ch in (0, 1):
                        o2 = bass.AP(tensor=qps_f.tensor,
                                     offset=qps_f.offset + ci * 512 + ch * 256,
                                     ap=[pstr, [2, 128]])
                        nc.tensor.transpose(o2, qkv[:, ch, c0:c0 + 128], id_s[:, :])
                    ksq = smpool.tile([128, 192], BF16, tag="ksq")
                    nc.vector.tensor_mul(ksq[:], kT2[:, ci, :], kT2[:, ci, :])
                    nc.tensor.matmul(kss_ps[:, :], ones_s[:, :], ksq[:],
                                     start=(gidx == 0 and ci == 0),
                                     stop=(gidx == n128_total // 2 - 1 and ci == 1))
                qT2 = smpool.tile([128, 2, 256], F8, tag="qT2")
                qps_v = bass.AP(tensor=qps_f.tensor, offset=qps_f.offset,
                                ap=[pstr, [512, 2], [256, 2], [2, 128]])
                qT2_v = qT2[:].rearrange("p c (h n) -> p c h n", h=2)
                nc.scalar.copy(out=qT2_v, in_=qps_v)
                nc.tensor.matmul(gramA[:, :], kT2[:, :, 0:128], qT2[:],
                                 start=(gidx == 0), stop=(gidx == n128_total // 2 - 1),
                                 perf_mode=DR)
                nc.tensor.matmul(gramB[:, :], kT2[:, :, 128:192], qT2[:],
                                 start=(gidx == 0), stop=(gidx == n128_total // 2 - 1),
                                 perf_mode=DR)

        qmv = statpool.tile([128, 2, 2], F32)
        vmv = statpool.tile([128, 2, 2], F32)
        nc.vector.memset(qmv[:], 0.0)
        nc.vector.memset(vmv[:], 0.0)
        nc.vector.bn_aggr(out=qmv[:, 0, :], in_=qstats[:, 0, :, :])
        nc.vector.bn_aggr(out=qmv[:, 1, :], in_=qstats[:, 1, :, :])
        nc.vector.bn_aggr(out=vmv[:, 0, :], in_=vstats[:, 0, :, :])
        nc.vector.bn_aggr(out=vmv[:64, 1, :], in_=vstats[:64, 1, :, :])
        nc.sync.dma_start(out=qstats_out[:], in_=qmv[:])
        nc.sync.dma_start(out=vstats_out[:], in_=vmv[:])
        gA = statpool.tile([128, 256], F32)
        gB = statpool.tile([64, 256], F32)
        kssb = statpool.tile([1, 192], F32)
        nc.scalar.copy(out=gA[:], in_=gramA[:])
        nc.scalar.copy(out=gB[:], in_=gramB[:])
        nc.scalar.copy(out=kssb[:], in_=kss_ps[:])
        nc.sync.dma_start(out=gram_out[0:128, :], in_=gA[:])
        nc.sync.dma_start(out=gram_out[128:192, :], in_=gB[:])
        nc.sync.dma_start(out=kss_out[:], in_=kssb[:])

    nc.compile()
    return nc


def build_l2(R=64, W=128):
    NS = R * W
    nc = bacc.Bacc("TRN2", target_bir_lowering=False, debug=False, num_devices=8)
    v_in = nc.dram_tensor("v_in", [128, 2 * NS], F8, kind="ExternalInput").ap()
    awT = nc.dram_tensor("awT", [128, 2, 256], F8, kind="ExternalInput").ap()
    pbias = nc.dram_tensor("pbias", [128, 2], F32, kind="ExternalInput").ap()
    sab = nc.dram_tensor("sab", [128, 2], F32, kind="ExternalInput").ap()  # 1/SA
    pbraw = nc.dram_tensor("pbraw", [128, 2], F32, kind="ExternalInput").ap()
    out = nc.dram_tensor("out", [128, 2, NS], BF16, kind="ExternalOutput").ap()

    with tile.TileContext(nc) as tc, ExitStack() as ctx:
        wpool = ctx.enter_context(tc.tile_pool(name="w", bufs=1))
        vpool = ctx.enter_context(tc.tile_pool(name="v", bufs=1))
        opool = ctx.enter_context(tc.tile_pool(name="o", bufs=1))
        pspool = ctx.enter_context(tc.tile_pool(name="ps", bufs=7, space="PSUM"))

        aw = wpool.tile([128, 2, 256], F8)
        nc.sync.dma_start(out=aw[:], in_=awT[:])
        pb = wpool.tile([128, 2], F32)
        nc.sync.dma_start(out=pb[:], in_=pbias[:])
        sav = wpool.tile([128, 2], F32)
        nc.sync.dma_start(out=sav[:], in_=sab[:])
        pbr = wpool.tile([128, 2], F32)
        nc.sync.dma_start(out=pbr[:], in_=pbraw[:])
        CH = 2048
        vts = []
        for ci in range(NS // CH):
            vtc = vpool.tile([128, 2, CH], F8, name=f"vt{ci}")
            q = (nc.sync, nc.gpsimd)[ci % 2]
            q.dma_start(out=vtc[:], in_=_dr3(v_in[:], ci * CH, NS, CH))
            vts.append(vtc)
        ot = opool.tile([128, 2, NS], BF16)

        # mb-outer: consecutive matmuls share the stationary operand
        for mb in range(2):
            msz = 128 if mb == 0 else 64
            for ic in range(NS // 512):
                c0 = ic * 512
                vtc = vts[c0 // CH]
                cc = c0 % CH
                ps = pspool.tile([128, 512], F32, tag="ps", name=f"ps{mb}_{ic}")
                nc.tensor.matmul(
                    ps[:, :],
                    aw[:, mb, :].rearrange("p (m two) -> p two m", two=2),
                    vtc[:, :, cc:cc + 512], start=True, stop=True, perf_mode=DRI)
                if ic % 2 == 0:
                    nc.vector.tensor_scalar(
                        out=ot[:msz, mb, c0:c0 + 512], in0=ps[:msz, :],
                        scalar1=pb[:msz, mb:mb + 1], scalar2=sav[:msz, mb:mb + 1],
                        op0=mybir.AluOpType.add, op1=mybir.AluOpType.mult)
                else:
                    nc.scalar.activation(
                        out=ot[:msz, mb, c0:c0 + 512], in_=ps[:msz, :],
                        func=mybir.ActivationFunctionType.Identity,
                        bias=pbr[:msz, mb:mb + 1], scale=sav[:msz, mb:mb + 1])
                if ic % 8 == 7:
                    h0 = (ic // 8) * 8 * 512
                    nc.sync.dma_start(out=out[:, mb, h0:h0 + 4096],
                                      in_=ot[:, mb, h0:h0 + 4096])
    nc.compile()
    return nc


WP = 130


def _dw_channel_at(pb, p):
    """kv channel (0..383) stored at partition p of dw pblock pb."""
    for (ipb, ib, opb, ob) in DWBLK:
        if opb == pb and ob <= p < ob + 64:
            return 128 * ipb + ib + (p - ob)
    return None


def prep_weights(w):
    """w: dict of reference weights (numpy f32). Returns dict of L1 input arrays."""
    out = {}
    qw = w["q_w"][:, :, 0, 0]          # (576, 192)
    qwT = np.zeros((128, 2, 768), np.float32)
    for (pb, h, x1b, qb, ch0, nch) in qkv_halves():
        win = 128 * pb + 64 * h
        qwT[0:128, 0, win:win + nch] = qw.T[0:128, ch0:ch0 + nch]
        qwT[0:64, 1, win:win + nch] = qw.T[128:192, ch0:ch0 + nch]
        qwT[64, 1, win:win + nch] = w["q_b"][ch0:ch0 + nch]
    qwTi = np.zeros((128, 6, 256), np.float32)
    for pb in range(6):
        qwTi[:, pb] = dri_pack(qwT[:, :, 128 * pb:128 * pb + 128])
    out["qwT"] = qf8(qwTi, SW)

    qdw = w["qdw_w"]                   # (576, 3, 3, 3) out, in-per-group, ky, kx
    qdwT = np.zeros((128, 54, 128), np.float32)
    for dy in (-1, 0, 1):
        for dx in (-1, 0, 1):
            ti = 3 * dy + dx + 4
            for (pb, h, x1b, qb, ch0, nch) in qkv_halves():
                for gl in range(nch // 3):
                    for i in range(3):
                        for j in range(3):
                            qdwT[x1b + 3 * gl + i, 6 * ti + pb, qb + 3 * gl + j] = \
                                qdw[ch0 + 3 * gl + j, i, dy + 1, dx + 1]
    out["qdw_wT"] = qdwT.astype(bf16)

    kvw = w["kv_w"]                    # (384, 192, 3, 3)
    # lo taps (128-ch rows) and packed hi taps, as DoubleRow pairs
    lo = np.zeros((9, 128, 384), np.float32)
    for dy in (-1, 0, 1):
        for dx in (-1, 0, 1):
            ti = 3 * dy + dx + 4
            lo[ti] = kvw[:, 0:128, dy + 1, dx + 1].T
    hi7 = np.zeros((128, 384), np.float32)
    hi7[0:64] = kvw[:, :, 2, 1].T[128:192]           # tap (1, 0) hi block
    kvp = np.zeros((4, 128, 384), np.float32)
    for j, dx in enumerate((-1, 0, 1)):
        kvp[j, 0:64] = kvw[:, :, 0, dx + 1].T[128:192]    # (-1, dx)
        kvp[j, 64:128] = kvw[:, :, 1, dx + 1].T[128:192]  # (0, dx)
    kvp[3, 0:64] = kvw[:, :, 2, 0].T[128:192]     # (1, -1)
    kvp[3, 64:128] = kvw[:, :, 2, 2].T[128:192]   # (1, +1)
    cwa = np.zeros((128, 7, 2, 384), np.float32)
    for i, (a, b) in enumerate([(0, 1), (2, 3), (4, 5), (6, 7)]):
        cwa[:, i, 0] = lo[a]
        cwa[:, i, 1] = lo[b]
    cwa[:, 4, 0] = lo[8]
    cwa[:, 4, 1] = hi7
    cwa[:, 5, 0] = kvp[0]
    cwa[:, 5, 1] = kvp[1]
    cwa[:, 6, 0] = kvp[2]
    cwa[:, 6, 1] = kvp[3]
    cwi = np.zeros((128, 7, 3, 256), np.float32)
    for i in range(7):
        for pb in range(3):
            cwi[:, i, pb] = dri_pack(cwa[:, i, :, 128 * pb:128 * pb + 128])
    out["cw"] = qf8(cwi, SW)

    kvdw = w["kvdw_w"][:, 0]           # (384, 3, 3)
    dwT = np.zeros((128, 54, 64), np.float32)
    for dy in (-1, 0, 1):
        for dx in (-1, 0, 1):
            ti = 3 * dy + dx + 4
            d = kvdw[:, dy + 1, dx + 1]
            for blk, (ipb, ib, opb, ob) in enumerate(DWBLK):
                ch0 = 128 * ipb + ib
                dwT[ib:ib + 64, 6 * ti + blk, :] = np.diag(d[ch0:ch0 + 64])
    out["kvdw_wT"] = dwT.astype(bf16)

    # newk: pass0 = (qkv pb2, qkv pb3) @ SKE/SQKV; pass1 = (dw pb0, dw pb1-hi) @ SKE/SDW8
    nk = w["newk_w"][:, :, 0, 0]       # (192, 384): in = [k(192) | k_mask(192)]
    nkm = np.zeros((128, 2, 2, 192), np.float32)
    for pl, pb in enumerate((2, 3)):
        for p in range(128):
            ch = qkv_channel_at(pb, p)
            if ch is not None:
                nkm[p, 0, pl] = nk[:, ch - 192]          # k part: qkv ch 192-383
    for pl, pb in enumerate((0, 1)):
        for p in range(128):
            ch = _dw_channel_at(pb, p)
            if ch is not None and ch < 192:
                nkm[p, 1, pl] = nk[:, 192 + ch]          # k_mask: dw ch 0-191
    nkm[:, 0] *= SKE / SQKV / SW
    nkm[:, 1] *= SKE / SDW8 / SW
    out["nkw"] = qf8(nkm, SW)

    nv = w["newv_w"][:, :, 0, 0]       # (192, 384): in = [v(192) | v_mask(192)]
    nvm = np.zeros((128, 2, 2, 192), np.float32)
    for pl, pb in enumerate((4, 5)):
        for p in range(128):
            ch = qkv_channel_at(pb, p)
            if ch is not None:
                nvm[p, 0, pl] = nv[:, ch - 384]          # v part: qkv ch 384-575
    for pl, pb in enumerate((1, 2)):
        for p in range(128):
            ch = _dw_channel_at(pb, p)
            if ch is not None and ch >= 192:
                nvm[p, 1, pl] = nv[:, ch]                # v_mask: dw ch 192-383
    nvm[:, 0] *= SKE / SQKV / SW
    nvm[:, 1] *= SKE / SDW8 / SW
    nvp = np.zeros((128, 2, 2, 2, 128), np.float32)
    nvp[:, :, :, 0, :] = nvm[:, :, :, 0:128]
    nvp[:, :, :, 1, 0:64] = nvm[:, :, :, 128:192]
    nvwi = np.zeros((128, 2, 2, 256), np.float32)
    for ps_ in range(2):
        for mb in range(2):
            nvwi[:, ps_, mb] = dri_pack(nvp[:, ps_, :, mb, :])
    out["nvw"] = qf8(nvwi, SW)

    out["ident"] = np.eye(128, dtype=e4m3)
    out["ones_col"] = np.ones((128, 1), dtype=bf16)

    x1b = np.zeros((128, 6), np.float32)
    qkvb = np.zeros((128, 6), np.float32)
    for (pb, h, x1b_base, qb, ch0, nch) in qkv_halves():
        x1b[x1b_base:x1b_base + nch, pb] = w["q_b"][ch0:ch0 + nch]
        qkvb[qb:qb + nch, pb] = w["qdw_b"][ch0:ch0 + nch]
    out["x1_bias"] = x1b * (SXI * SW)
    out["qkv_bias"] = qkvb * SQKV

    kvb = np.zeros((128, 3), np.float32)
    kvb[:, 0] = w["kv_b"][0:128]
    kvb[0:64, 1] = w["kv_b"][128:192]
    kvb[64:128, 1] = w["kv_b"][192:256]
    kvb[:, 2] = w["kv_b"][256:384]
    out["kv_bias"] = kvb * (SXI * SW)
    dwb = np.zeros((128, 3), np.float32)
    for (ipb, ib, opb, ob) in DWBLK:
        dwb[ob:ob + 64, opb] = w["kvdw_b"][128 * ipb + ib:128 * ipb + ib + 64]
    out["kvdw_bias"] = dwb * SDW8
    out["newk_b_row"] = w["newk_b"][None, :].astype(np.float32) * SK8
    nvb = np.zeros((128, 2), np.float32)
    nvb[:, 0] = w["newv_b"][0:128]
    nvb[0:64, 1] = w["newv_b"][128:192]
    out["newv_bias"] = nvb * SV8
    return out


def prep_masks(R, H, half):
    m = np.zeros((R + 2, WP), np.float32)
    for r in range(R + 2):
        g = half * R + (r - 1)
        if 0 <= g < H:
            m[r, 1:129] = MINV
    return m.reshape(1, -1)


def prep_core(x, xm, b, half, R, H):
    xp = np.zeros((192, R + 2, WP), np.float32)
    mp = np.zeros((192, R + 4, WP), np.float32)
    for r in range(R + 2):
        g = half * R + (r - 1)
        if 0 <= g < H:
            xp[:, r, 1:129] = x[b, :, g, :]
    for r in range(R + 4):
        g = half * R + (r - 2)
        if 0 <= g < H:
            mp[:, r, 1:129] = xm[b, :, g, :]
    xp = xp.reshape(192, -1)
    xcb = np.zeros((128, 2, xp.shape[1]), np.float32)
    xcb[:, 0] = xp[0:128]
    xcb[0:64, 1] = xp[128:192]
    ind = np.zeros((R + 2, WP), np.float32)
    for r in range(R + 2):
        if 0 <= half * R + (r - 1) < H:
            ind[r, 1:129] = 1.0
    xcb[64, 1] = ind.reshape(-1)
    mp = mp.reshape(192, -1)
    L = mp.shape[1]
    hi = mp[128:192]
    d1 = np.zeros((128, L), np.float32)
    d2 = np.zeros((128, L), np.float32)
    d1[0:64] = hi
    d1[64:128, :L - 130] = hi[:, 130:]
    d2[0:64] = hi
    d2[64:128, :L - 2] = hi[:, 2:]
    return {
        "xc": qf8(xcb, SXI),
        "xm_lo": qf8(mp[0:128], SXI),
        "xm_d1": qf8(d1, SXI), "xm_d2": qf8(d2, SXI),
        "mask_rc": prep_masks(R, H, half).astype(bf16),
    }


def _q_maps():
    """q channel c (0..191) -> (pblock 0/1, partition)."""
    part = np.zeros(192, np.int64)
    pblk = np.zeros(192, np.int64)
    for (pb, h, x1b, qb, ch0, nch) in qkv_halves():
        if pb >= 2:
            continue
        for i in range(nch):
            pblk[ch0 + i] = pb
            part[ch0 + i] = qb + i
    return pblk, part


def _ss_from_qstats(stats, n_half):
    pblk, part = _q_maps()
    mv = stats.astype(np.float64)
    return (mv[part, pblk, 1] + mv[part, pblk, 0] ** 2) * n_half


def _ss_from_vstats(stats, n_half):
    ss = np.zeros(192, np.float64)
    mv = stats.astype(np.float64)
    ss[0:128] = (mv[0:128, 0, 1] + mv[0:128, 0, 0] ** 2) * n_half
    ss[128:192] = (mv[0:64, 1, 1] + mv[0:64, 1, 0] ** 2) * n_half
    return ss


def glue(res0, res1, temperature, proj_w, proj_b, n_half):
    """Combine two half-core L1 results -> L2 inputs (awT fp8, pbias, sa)."""
    G = res0["gram_out"].astype(np.float64) + res1["gram_out"].astype(np.float64)
    pblk, part = _q_maps()
    qcol = pblk * 128 + part
    G = G[:, qcol]                              # (d, c): sum_n k[d,n] q[c,n]
    qss = _ss_from_qstats(res0["qstats_out"], n_half) + _ss_from_qstats(res1["qstats_out"], n_half)
    vss = _ss_from_vstats(res0["vstats_out"], n_half) + _ss_from_vstats(res1["vstats_out"], n_half)
    kss = (res0["kss_out"].astype(np.float64) + res1["kss_out"].astype(np.float64))[0]
    qn = np.maximum(np.sqrt(qss), 1e-12)
    kn = np.maximum(np.sqrt(kss), 1e-12)
    vn = np.maximum(np.sqrt(vss), 1e-12)
    A = G.T / (qn[:, None] * kn[None, :])      # (c, d)
    M = np.zeros((192, 192), np.float64)
    t = np.asarray(temperature).reshape(-1)
    for h in range(8):
        sl = slice(24 * h, 24 * h + 24)
        a = A[sl, sl] * t[h]
        a = a - a.max(axis=-1, keepdims=True)
        e = np.exp(a)
        sm = e / e.sum(axis=-1, keepdims=True)
        M[sl, sl] = sm / vn[None, sl]
    At = proj_w[:, :, 0, 0].astype(np.float64) @ M   # (out-ch o, d)
    SA = 2.0 ** np.floor(np.log2(128.0 / max(np.abs(At).max(), 1e-30)))
    awT = np.zeros((128, 2, 192), np.float32)
    awT[:, 0, :] = At.T[0:128]
    awT[0:64, 1, :] = At.T[128:192]
    awp = np.zeros((128, 2, 2, 128), np.float32)
    awp[:, :, 0, :] = awT[:, :, 0:128]
    awp[:, :, 1, 0:64] = awT[:, :, 128:192]
    awTi = np.zeros((128, 2, 256), np.float32)
    for mb in range(2):
        awTi[:, mb] = dri_pack(awp[:, :, mb, :])
    pbias = np.zeros((128, 2), np.float32)
    pbias[:, 0] = proj_b[0:128]
    pbias[0:64, 1] = proj_b[128:192]
    return {"awT": qf8(awTi, SA), "pbias": pbias * SA, "pbraw": pbias,
            "sab": np.full((128, 2), 1.0 / SA, np.float32)}


def _prep_vin(v_out):
    """L1 v_out [128, 2, NS] -> L2 v_in [128, 2*NS], pad rows zeroed."""
    vv = np.array(v_out)
    vv[64:128, 1, :] = np.zeros(1, e4m3)
    return vv.reshape(128, -1)


# ---------------- driver: kernel(**inputs) ----------------
from concourse.bass_utils import run_bass_kernel_spmd

R_FULL, H_FULL, B_FULL = 64, 128, 4
_NC1 = None
_NC2 = None


def _get_progs():
    global _NC1, _NC2
    if _NC1 is None:
        _NC1 = build_l1(R=R_FULL, S=16)
        _NC2 = build_l2(R=R_FULL)
    return _NC1, _NC2


def kernel(**inputs):
    inputs = {k: np.asarray(v) for k, v in inputs.items()}
    x, xm = inputs["x"], inputs["x_mask"]
    nc1, nc2 = _get_progs()
    wprep = prep_weights(inputs)
    in_maps = []
    for core in range(8):
        b, half = core // 2, core % 2
        m = dict(wprep)
        m.update(prep_core(x, xm, b, half, R_FULL, H_FULL))
        in_maps.append(m)
    res1 = run_bass_kernel_spmd(nc1, in_maps, list(range(8))).results

    n_half = R_FULL * 128
    in_maps2 = []
    for core in range(8):
        b, half = core // 2, core % 2
        if half == 0:
            l2c = glue(res1[2 * b], res1[2 * b + 1], inputs["temperature"],
                       inputs["proj_w"], inputs["proj_b"], n_half)
        m = dict(l2c)
        m["v_in"] = _prep_vin(res1[core]["v_out"])
        in_maps2.append(m)
    res2 = run_bass_kernel_spmd(nc2, in_maps2, list(range(8))).results

    out = np.empty((B_FULL, 192, H_FULL, 128), np.float32)
    for core in range(8):
        b, half = core // 2, core % 2
        o = np.asarray(res2[core]["out"]).astype(np.float32)   # [128, 2, NS] bf16
        sl = out[b, :, half * R_FULL:(half + 1) * R_FULL, :]
        sl[0:128] = o[:, 0, :].reshape(128, R_FULL, 128)
        sl[128:192] = o[0:64, 1, :].reshape(64, R_FULL, 128)
    return out
